# revision 8
# baseline (speedup 1.0000x reference)
"""Trainium2 Bass kernel for GroupedQueryAttention (sliding-window + global).

Sharding: 8 cores = 2 (batch) x 4 (GQA groups). Core c handles batch c//4 and
kv-head g=c%4 with its 4 query heads. Wq/Wk/Wv column-sharded, Wo row-sharded;
each core emits outT = (context_g @ Wo_g)^T in bf16; the host transposes,
upcasts and sums partials per batch.

v2 design notes (vs baseline):
- Host pre-transposes and pre-casts x (xT in fp8) so the kernel does no
  x transposes and no f32->bf16 casts on device.
- QKV projection and output projection run as fp8 DoubleRow matmuls
  (K=256 per instruction).
- Scalar engine runs only {exp, ln}-table functions (softmax exp, and the
  L2 norm via rsqrt(x) = exp(-0.5*ln(x))), so the activation table is
  loaded once -- no 1.3us table swaps.
- Softmax reciprocal on [1,512] before the partition broadcast (not after).
- Per-tile work is software-pipelined A(i) | scores(i-1) | ctx(i-2) so the
  tensor engine stays continuously busy and ramps to its 2.4GHz p-state.
"""

import sys

for _p in (
    "/opt/trn_rl_repo",
    "/root/.axon_site",
    "/root/.axon_site/_ro/pypackages",
    "/root/.axon_site/_ro/trn_rl_repo",
):
    if _p not in sys.path:
        sys.path.insert(0, _p)

from contextlib import ExitStack

import numpy as np

import concourse.bass as bass  # noqa: F401  (registers engine classes)
import concourse.tile as tile
from concourse import bacc, mybir
from concourse.bass_utils import run_bass_kernel_spmd
from concourse.masks import make_identity

B, S, DM = 2, 2048, 1024
NH, NKV, DH = 16, 4, 64
HPC = 4  # q heads per core (one full GQA group)
WINDOW, NGLOB = 256, 4
SCALE = 1.0 / np.sqrt(DH)
CAP = 15.0
EPS = 1e-8
P = 128
NT = S // P  # 16 sequence tiles
G = HPC + 1  # 4 q heads + 1 k head share L2norm/RoPE processing
F32 = mybir.dt.float32
BF16 = mybir.dt.bfloat16
FP8 = mybir.dt.float8e4
MULT = mybir.AluOpType.mult
EXP = mybir.ActivationFunctionType.Exp
LN = mybir.ActivationFunctionType.Ln
DR = mybir.MatmulPerfMode.DoubleRow

USE_FP8_QKV = False
USE_FP8_WO = False


def _build_kernel(ctx, tc, d):
    nc = tc.nc
    xdt = FP8 if USE_FP8_QKV else BF16
    wodt = FP8 if USE_FP8_WO else BF16

    consts = ctx.enter_context(tc.tile_pool(name="consts", bufs=1))
    ident = consts.tile([P, P], F32)
    make_identity(nc, ident[:])
    ident_bf = consts.tile([P, P], BF16)
    nc.vector.tensor_copy(ident_bf[:], ident[:])

    # resident inputs
    wqkv_sb = consts.tile([P, 8, 384], xdt)
    nc.sync.dma_start(wqkv_sb[:], d["wqkv"].rearrange("(c p) n -> p c n", p=P))
    wo_sb = consts.tile([P, 2, DM], wodt)
    nc.sync.dma_start(wo_sb[:], d["wo"].rearrange("(c p) n -> p c n", p=P))
    cos2_sb = consts.tile([P, NT, DH], BF16)
    nc.sync.dma_start(cos2_sb[:], d["cos2"].rearrange("(t p) n -> p t n", p=P))
    sin2_sb = consts.tile([P, NT, DH], BF16)
    nc.sync.dma_start(sin2_sb[:], d["sin2"].rearrange("(t p) n -> p t n", p=P))

    # xT resident [128, 8(dm chunk), 2048(s)], DMA'd per s-tile for pipelining
    xts = consts.tile([P, 8, S], xdt)
    # persistent per-tile q^T/k^T/v tensors
    qt_all = consts.tile([64, NT, HPC * P], BF16)  # [dq, t, (h,q)]
    kt_all = consts.tile([64, NT, P], BF16)
    v_all = consts.tile([P, NT, 65], BF16)
    # context accumulators: [128 = dq of head pair, c(pair), s-chunk] per sc
    ctxt = [consts.tile([P, 2, 512], wodt, name=f"ctxt_{sc}") for sc in range(4)]

    for i in range(NT):
        nc.gpsimd.memset(v_all[:, i, 64:65], 1.0)

    work = ctx.enter_context(tc.tile_pool(name="work", bufs=3))
    attn = ctx.enter_context(tc.tile_pool(name="attn", bufs=3))
    outp = ctx.enter_context(tc.tile_pool(name="outp", bufs=4))
    mbp = ctx.enter_context(tc.tile_pool(name="mbp", bufs=3))

    ps = ctx.enter_context(tc.tile_pool(name="ps", bufs=1, space="PSUM"))

    # ---- software pipelined main loop -----------------------------------
    # iter k: DMA x(k+2)/band(k), A-part1 (QKV+norm+rope) for tile k,
    #          scores+exp+mask for tile k-1, ctx+den+div for tile k-2.
    def dma_x(i):
        if not (0 <= i < NT):
            return
        nc.sync.dma_start(
            xts[:, :, P * i : P * (i + 1)],
            d["xT"].rearrange("(c p) s -> p c s", p=P)[
                :, :, P * i : P * (i + 1)
            ],
        )

    band_tiles = [None] * NT

    def dma_band(t):
        if not (0 <= t < NT):
            return
        mb = mbp.tile([P, 4, P], BF16, name=f"mb_{t}", tag="mb")
        nc.sync.dma_start(mb[:], d["band"][t])
        band_tiles[t] = mb

    # block lists per score-tile: (kt, j) pairs; j = block slot 0..3
    def blocks_of(t):
        bl = [(kt, kt - (t - 2)) for kt in range(max(0, t - 2), t + 1)]
        if t >= 3:
            bl.append((0, 3))  # global block: full k-tile 0, mask rows 4+ zero
        return bl

    qkv_ps = [None] * NT  # psum tile of QKV output per tile (live ~1 iter)
    rsqe_t = [None] * NT
    rp_t = [None] * NT
    ex_t = {}  # (t, pass) -> exp'd tile
    em_t = {}
    pcx_t = [None] * NT
    rc_t = [None] * NT

    dma_x(0)
    dma_x(1)
    dma_band(0)

    for k in range(NT + 2):
        i = k  # A-tile
        t1 = k - 1  # score tile
        t2 = k - 2  # ctx tile

        dma_x(i + 2)
        dma_band(t1 + 1)

        # ---------------- PE: QKV(i) ---------------------------------
        if i < NT:
            pq = ps.tile([P, 384], F32, name=f"pq_{i}", tag="qkv", bufs=2,
                         padded_shape=[P, 512])
            if USE_FP8_QKV:
                for j2 in range(4):
                    nc.tensor.matmul(
                        pq[:],
                        lhsT=xts[:, 2 * j2 : 2 * j2 + 2, P * i : P * (i + 1)],
                        rhs=wqkv_sb[:, 2 * j2 : 2 * j2 + 2, :],
                        start=(j2 == 0),
                        stop=(j2 == 3),
                        perf_mode=DR,
                    )
            else:
                for mj in range(8):
                    nc.tensor.matmul(
                        pq[:],
                        lhsT=xts[:, mj, P * i : P * (i + 1)],
                        rhs=wqkv_sb[:, mj, :],
                        start=(mj == 0),
                        stop=(mj == 7),
                    )
            qkv_ps[i] = pq

        # ---------------- PE: scores(t1) two passes -------------------
        if 0 <= t1 < NT:
            bl = blocks_of(t1)
            p1 = [b for b in bl if b[1] < 2]
            p2 = [b for b in bl if b[1] >= 2]
            qrhs = qt_all[:, t1, :].rearrange("p (h q) -> p h q", h=HPC)
            for pi, blkpass in enumerate((p1, p2)):
                if not blkpass:
                    continue
                sc_ps = ps.tile(
                    [P, 2, 512], F32, name=f"sc_{t1}_{pi}", tag="sc", bufs=2
                )
                for bj, (kt, j) in enumerate(blkpass):
                    if j == 3:
                        lhs = kt_all[:, 0, :]
                    else:
                        lhs = kt_all[:, kt, :]
                    nc.tensor.matmul(
                        sc_ps[:, bj, :], lhsT=lhs, rhs=qrhs,
                        start=True, stop=True,
                    )
                nb = len(blkpass)
                # exp on scalar engine (one table, never swapped)
                ex = attn.tile([P, 2, 512], BF16, name=f"ex_{t1}_{pi}", tag="ex")
                nc.scalar.activation(
                    ex[:, 0:nb, :], sc_ps[:, 0:nb, :], EXP, scale=SCALE
                )
                # mask multiply on DVE (bf16 sbuf-sbuf, fast mode)
                em = attn.tile([P, 2, 512], BF16, name=f"em_{t1}_{pi}", tag="em")
                mb = band_tiles[t1]
                jlist = [j for (_, j) in blkpass]
                nc.gpsimd.tensor_tensor(
                    em[:, 0:nb, :].rearrange("p b (h q) -> p b h q", h=HPC),
                    ex[:, 0:nb, :].rearrange("p b (h q) -> p b h q", h=HPC),
                    mb[:, jlist[0] : jlist[0] + nb, :]
                    .unsqueeze(2)
                    .broadcast_to([P, nb, HPC, P]),
                    op=MULT,
                )
                ex_t[(t1, pi)] = ex
                em_t[(t1, pi)] = em

        # ---------------- DVE/Act/Pool: norm + rope for tile i --------
        if i < NT:
            pq = qkv_ps[i]
            ssq = work.tile([P, G * DH], F32, tag="ssq")
            nc.scalar.square(ssq[:], pq[:, 0 : G * DH])
            red = work.tile([P, G], F32, tag="red")
            nc.vector.tensor_reduce(
                red[:],
                ssq[:].rearrange("p (g n) -> p g n", g=G),
                axis=mybir.AxisListType.X,
                op=mybir.AluOpType.add,
            )
            lss = work.tile([P, G], F32, tag="lss")
            nc.scalar.activation(lss[:], red[:], LN)
            rsqe = work.tile([P, G], F32, tag="rsqe")
            nc.scalar.activation(rsqe[:], lss[:], EXP, scale=-0.5)
            rsqe_t[i] = rsqe
            # q,k normalize (also moves PSUM -> SBUF)
            qkn = work.tile([P, G * DH], BF16, tag="qkn")
            nc.vector.tensor_tensor(
                qkn[:].rearrange("p (g n) -> p g n", g=G),
                pq[:, 0 : G * DH].rearrange("p (g n) -> p g n", g=G),
                rsqe[:].unsqueeze(-1).broadcast_to([P, G, DH]),
                op=MULT,
            )
            # v copy out of psum
            nc.scalar.copy(v_all[:, i, 0:64], pq[:, 320:384])

            # RoPE: t1v = qkn*[c|c], t2v = qkn*[s|s];
            # out[:32] = t1[:32]-t2[32:], out[32:] = t2[:32]+t1[32:]
            qv = qkn[:].rearrange("p (g n) -> p g n", g=G)
            cb = cos2_sb[:, i, :].unsqueeze(1).broadcast_to([P, G, DH])
            sb = sin2_sb[:, i, :].unsqueeze(1).broadcast_to([P, G, DH])
            ta = work.tile([P, G * DH], BF16, tag="ta")
            tb = work.tile([P, G * DH], BF16, tag="tb")
            tav = ta[:].rearrange("p (g n) -> p g n", g=G)
            tbv = tb[:].rearrange("p (g n) -> p g n", g=G)
            nc.gpsimd.tensor_tensor(tav, qv, cb, op=MULT)
            nc.gpsimd.tensor_tensor(tbv, qv, sb, op=MULT)
            rp = work.tile([P, G * DH], BF16, tag="rp")
            rv = rp[:].rearrange("p (g n) -> p g n", g=G)
            nc.gpsimd.tensor_tensor(
                rv[:, :, 0:32], tav[:, :, 0:32], tbv[:, :, 32:64],
                op=mybir.AluOpType.subtract,
            )
            nc.gpsimd.tensor_tensor(
                rv[:, :, 32:64], tbv[:, :, 0:32], tav[:, :, 32:64],
                op=mybir.AluOpType.add,
            )
            rp_t[i] = rp

            # PE: transposes of roped q (2 blocks) and k
            qkT = ps.tile([P, 3, P], BF16, name=f"qkT_{i}", tag="qkv", bufs=2,
                          padded_shape=[P, 4, P])
            for hp in range(2):
                nc.tensor.transpose(
                    qkT[:, hp, :], rp[:, P * hp : P * (hp + 1)], ident_bf[:]
                )
            nc.tensor.transpose(qkT[0:64, 2, :], rp[:, 256:320], ident_bf[:])
            # Pool copies PSUM -> SBUF persistent tiles
            # heads: qkT[:, hp, :]: partitions 0-63 = head 2hp, 64-127 = head 2hp+1
            nc.vector.tensor_copy(
                qt_all[:, i, :].rearrange("p (h q) -> p h q", h=HPC)[:, 0::2, :],
                qkT[0:64, 0:2, :],
            )
            nc.vector.tensor_copy(
                qt_all[:, i, :].rearrange("p (h q) -> p h q", h=HPC)[:, 1::2, :],
                qkT[64:128, 0:2, :],
            )
            nc.vector.tensor_copy(kt_all[:, i, :], qkT[0:64, 2, :])

        # ---------------- PE: ctx(t2) + denominators ------------------
        if 0 <= t2 < NT:
            bl = blocks_of(t2)
            pcx = ps.tile([P, 512], F32, name=f"pcx_{t2}", tag="cx", bufs=2)
            n_all = len(bl)
            bi = 0
            for pi in (0, 1):
                blkpass = [b for b in bl if (b[1] < 2) == (pi == 0)]
                if not blkpass:
                    continue
                em = em_t.pop((t2, pi))
                ex_t.pop((t2, pi), None)
                for bj, (kt, j) in enumerate(blkpass):
                    nc.tensor.matmul(
                        pcx[0:65, :],
                        lhsT=v_all[:, kt, :],
                        rhs=em[:, bj, :],
                        start=(bi == 0),
                        stop=(bi == n_all - 1),
                    )
                    bi += 1
            pcx_t[t2] = pcx
            # reciprocal of denominators (row 64), then broadcast
            rc = attn.tile([1, 512], F32, tag="rc")
            nc.vector.reciprocal(rc[:], pcx[64:65, :])
            rcb = attn.tile([64, 512], F32, tag="rcb")
            nc.gpsimd.partition_broadcast(rcb[:], rc[:])
            rc_t[t2] = rcb

        # ---------------- DVE: divide & store ctx(t3) -----------------
        t3 = t2  # same iter, after broadcast
        if 0 <= t3 < NT:
            pcx = pcx_t[t3]
            rcb = rc_t[t3]
            sc_, qoff = t3 // 4, (t3 % 4) * P
            # heads (0,2) -> partitions 0:64 of c=0,1 ; heads (1,3) -> 64:128
            for half in range(2):
                nc.vector.tensor_tensor(
                    ctxt[sc_][64 * half : 64 * half + 64, :, qoff : qoff + P],
                    pcx[0:64, :]
                    .rearrange("p (h q) -> p h q", h=HPC)[:, half::2, :],
                    rcb[:].rearrange("p (h q) -> p h q", h=HPC)[:, half::2, :],
                    op=MULT,
                )
            pcx_t[t3] = None
            rc_t[t3] = None

    # ---------------- Phase C: output projection (transposed) ------------
    for sc in range(4):
        for mo in range(8):
            po = ps.tile([P, 512], F32, name=f"po_{sc}_{mo}", tag="qkv", bufs=2,
                         padded_shape=[P, 512])
            if USE_FP8_WO:
                nc.tensor.matmul(
                    po[:],
                    lhsT=wo_sb[:, :, P * mo : P * (mo + 1)],
                    rhs=ctxt[sc][:],
                    start=True,
                    stop=True,
                    perf_mode=DR,
                )
            else:
                for c in range(2):
                    nc.tensor.matmul(
                        po[:],
                        lhsT=wo_sb[:, c, P * mo : P * (mo + 1)],
                        rhs=ctxt[sc][:, c, :],
                        start=(c == 0),
                        stop=(c == 1),
                    )
            ob = outp.tile([P, 512], BF16, tag="ob")
            if mo % 2 == 1:
                nc.scalar.copy(ob[:], po[:])
            else:
                nc.vector.tensor_copy(ob[:], po[:])
            nc.sync.dma_start(
                d["outT"][P * mo : P * (mo + 1), 512 * sc : 512 * (sc + 1)], ob[:]
            )


def build_program():
    nc = bacc.Bacc("TRN2", target_bir_lowering=False, debug=False, num_devices=8)
    xdt = FP8 if USE_FP8_QKV else BF16
    wodt = FP8 if USE_FP8_WO else BF16
    d = {}
    d["xT"] = nc.dram_tensor("xT", [DM, S], xdt, kind="ExternalInput").ap()
    d["wqkv"] = nc.dram_tensor("wqkv", [DM, 384], xdt, kind="ExternalInput").ap()
    d["wo"] = nc.dram_tensor("wo", [256, DM], wodt, kind="ExternalInput").ap()
    d["cos2"] = nc.dram_tensor("cos2", [S, DH], BF16, kind="ExternalInput").ap()
    d["sin2"] = nc.dram_tensor("sin2", [S, DH], BF16, kind="ExternalInput").ap()
    d["band"] = nc.dram_tensor("band", [NT, P, 4, P], BF16, kind="ExternalInput").ap()
    d["outT"] = nc.dram_tensor("outT", [DM, S], BF16, kind="ExternalOutput").ap()
    with tile.TileContext(nc) as tc, ExitStack() as ctx:
        _build_kernel(ctx, tc, d)
    nc.compile()
    return nc


def make_masks(mask_np):
    """Pack the combined (caller mask & sliding-window|global) mask into
    [k, q]-oriented band tiles; block j=0..2 is k-tile t-2+j, block 3 is the
    global block (k-tile 0, only rows < NGLOB can be nonzero, t>=3 only)."""
    mask_np = np.asarray(mask_np).astype(bool)
    q = np.arange(S)[:, None]
    k = np.arange(S)[None, :]
    wmask = ((k <= q) & (k > q - WINDOW)) | (k < NGLOB)
    combT = (mask_np[0, 0] & wmask).T.astype(np.float32)  # [k, q]
    band = np.zeros((NT, P, 4, P), np.float32)
    for t in range(NT):
        for kt in range(max(0, t - 2), t + 1):
            j = kt - (t - 2)
            band[t, :, j, :] = combT[P * kt : P * (kt + 1), P * t : P * (t + 1)]
        if t >= 3:
            band[t, 0:NGLOB, 3, :] = combT[0:NGLOB, P * t : P * (t + 1)]
    return band


def make_in_maps(x, cos, sin, mask, Wq, Wk, Wv, Wo):
    import ml_dtypes

    bf = ml_dtypes.bfloat16
    f8 = ml_dtypes.float8_e4m3
    xdt = f8 if USE_FP8_QKV else bf
    wodt = f8 if USE_FP8_WO else bf
    x = np.asarray(x, np.float32)
    cos = np.asarray(cos, np.float32)
    sin = np.asarray(sin, np.float32)
    cos2 = np.concatenate([cos, cos], axis=1).astype(bf)
    sin2 = np.concatenate([sin, sin], axis=1).astype(bf)
    Wq, Wk, Wv = (np.asarray(a, np.float32) for a in (Wq, Wk, Wv))
    Wo = np.asarray(Wo, np.float32).astype(wodt)
    band = make_masks(mask).astype(bf)
    in_maps = []
    xT = [np.ascontiguousarray(x[b].T).astype(xdt) for b in range(B)]
    for c in range(8):
        b, g = divmod(c, 4)
        wqkv = np.concatenate(
            [
                Wq[:, 256 * g : 256 * (g + 1)],
                Wk[:, 64 * g : 64 * (g + 1)],
                Wv[:, 64 * g : 64 * (g + 1)],
            ],
            axis=1,
        ).astype(xdt)
        in_maps.append(
            {
                "xT": xT[b],
                "wqkv": np.ascontiguousarray(wqkv),
                "wo": np.ascontiguousarray(Wo[256 * g : 256 * (g + 1), :]),
                "cos2": cos2,
                "sin2": sin2,
                "band": band,
            }
        )
    return in_maps


_PROGRAM = None


def _get_program():
    global _PROGRAM
    if _PROGRAM is None:
        _PROGRAM = build_program()
    return _PROGRAM


def kernel(x, cos, sin, mask, Wq, Wk, Wv, Wo, _trace=False, _trace_kwargs=None):
    nc = _get_program()
    in_maps = make_in_maps(x, cos, sin, mask, Wq, Wk, Wv, Wo)
    res = run_bass_kernel_spmd(
        nc, in_maps, list(range(8)), trace=_trace, **(_trace_kwargs or {})
    )
    out = np.zeros((B, S, DM), np.float32)
    for c in range(8):
        out[c // 4] += res.results[c]["outT"].T.astype(np.float32)
    if _trace:
        kernel._last_results = res
    return out


# revision 9
# speedup vs baseline: 1.1778x; 1.1778x over previous
"""Trainium2 Bass kernel for GroupedQueryAttention (sliding-window + global).

Sharding: 8 cores = 2 (batch) x 4 (GQA groups). Core c handles batch c//4 and
kv-head g=c%4 with its 4 query heads. Wq/Wk/Wv column-sharded, Wo row-sharded;
each core emits outT = (context_g @ Wo_g)^T in bf16; the host transposes,
upcasts and sums partials per batch.

v3 design notes:
- Host pre-transposes and pre-casts x to bf16 (xT): no device-side x
  transposes or f32->bf16 casts.
- Scalar engine uses only {Exp, Square, Copy} (all live in the exp
  activation table): zero ACT_TABLE_LOAD swaps. The L2-norm rsqrt and the
  softmax-denominator reciprocal run as DVE Newton iterations (bit-trick
  seed for rsqrt; host-provided 1/attended-count seed for the denominator,
  avoiding the ~3.3us hardware RECIPROCAL).
- Deep software pipeline: iter k = QKV(k) | transposes(k-1) |
  scores+exp+mask(k-2) | ctx(k-3) | recip+divide(k-4), giving every
  cross-engine dependency >= 1 iteration of slack so the tensor engine
  streams continuously and ramps to its fast p-state.
- PSUM budget: qkv tag 2 banks + one 4-bank score tile + ctx 2 banks = 8.
"""

import sys

for _p in (
    "/opt/trn_rl_repo",
    "/root/.axon_site",
    "/root/.axon_site/_ro/pypackages",
    "/root/.axon_site/_ro/trn_rl_repo",
):
    if _p not in sys.path:
        sys.path.insert(0, _p)

from contextlib import ExitStack

import numpy as np

import concourse.bass as bass  # noqa: F401  (registers engine classes)
import concourse.tile as tile
from concourse import bacc, mybir
from concourse.bass_utils import run_bass_kernel_spmd
from concourse.masks import make_identity

B, S, DM = 2, 2048, 1024
NH, NKV, DH = 16, 4, 64
HPC = 4
WINDOW, NGLOB = 256, 4
SCALE = 1.0 / np.sqrt(DH)
CAP = 15.0
EPS = 1e-8
P = 128
NT = S // P
G = HPC + 1
F32 = mybir.dt.float32
U32 = mybir.dt.uint32
BF16 = mybir.dt.bfloat16
MULT = mybir.AluOpType.mult
ADD = mybir.AluOpType.add
SUB = mybir.AluOpType.subtract
EXP = mybir.ActivationFunctionType.Exp
RSQRT_MAGIC = float(np.frombuffer(np.uint32(0x5F3759DF).tobytes(), np.float32)[0])


def _build_kernel(ctx, tc, d):
    nc = tc.nc

    consts = ctx.enter_context(tc.tile_pool(name="consts", bufs=1))
    ident_bf = consts.tile([P, P], BF16)
    identf = consts.tile([P, P], F32)
    make_identity(nc, identf[:])
    nc.vector.tensor_copy(ident_bf[:], identf[:])
    magic = consts.tile([P, 1], F32)
    nc.vector.memset(magic[:], RSQRT_MAGIC)

    wqkv_sb = consts.tile([P, 8, 384], BF16)
    nc.sync.dma_start(wqkv_sb[:], d["wqkv"].rearrange("(c p) n -> p c n", p=P))
    wo_sb = consts.tile([P, 2, DM], BF16)
    nc.sync.dma_start(wo_sb[:], d["wo"].rearrange("(c p) n -> p c n", p=P))
    cos2_sb = consts.tile([P, NT, DH], BF16)
    nc.sync.dma_start(cos2_sb[:], d["cos2"].rearrange("(t p) n -> p t n", p=P))
    sin2_sb = consts.tile([P, NT, DH], BF16)
    nc.sync.dma_start(sin2_sb[:], d["sin2"].rearrange("(t p) n -> p t n", p=P))
    g_sb = consts.tile([1, NT, 512], BF16)
    nc.sync.dma_start(g_sb[:], d["gden"].unsqueeze(0))

    xts = consts.tile([P, 8, S], BF16)
    qkt_all = consts.tile([64, NT, 5 * P], BF16)  # 4 q heads then k, per tile
    v_all = consts.tile([P, NT, 65], BF16)
    ctxt = [consts.tile([P, 2, 512], BF16, name=f"ctxt_{sc}") for sc in range(4)]

    for i in range(NT):
        nc.gpsimd.memset(v_all[:, i, 64:65], 1.0)

    work = ctx.enter_context(tc.tile_pool(name="work", bufs=3))
    attn = ctx.enter_context(tc.tile_pool(name="attn", bufs=3))
    denp = ctx.enter_context(tc.tile_pool(name="denp", bufs=3))
    outp = ctx.enter_context(tc.tile_pool(name="outp", bufs=4))
    mbp = ctx.enter_context(tc.tile_pool(name="mbp", bufs=4))

    ps = ctx.enter_context(tc.tile_pool(name="ps", bufs=1, space="PSUM"))

    def dma_x(i):
        if 0 <= i < NT:
            nc.sync.dma_start(
                xts[:, :, P * i : P * (i + 1)],
                d["xT"].rearrange("(c p) s -> p c s", p=P)[:, :, P * i : P * (i + 1)],
            )

    band_tiles = [None] * NT

    def dma_band(t):
        if 0 <= t < NT:
            mb = mbp.tile([P, 4, P], BF16, name=f"mb_{t}", tag="mb")
            nc.sync.dma_start(mb[:], d["band"][t])
            band_tiles[t] = mb

    def blocks_of(t):
        bl = [(kt, kt - (t - 2)) for kt in range(max(0, t - 2), t + 1)]
        if t >= 3:
            bl.append((0, 3))
        return bl

    qkv_ps = [None] * NT
    ssq_t = [None] * NT
    rp_t = [None] * NT
    qkT_t = [None] * NT
    ex_t = [None] * NT
    em_t = [None] * NT
    sc_ps_t = [None] * NT
    pcx_t = [None] * NT
    den_t = [None] * NT
    rcb_t = [None] * NT

    dma_x(0)
    dma_x(1)

    for k in range(NT + 4):
        iA = k       # QKV + norm + rope
        iT = k - 1   # transposes + qkT copies
        t1 = k - 2   # scores + exp + mask
        t2 = k - 3   # ctx matmuls + den extract
        t3 = k - 4   # reciprocal + divide

        dma_x(iA + 2)
        dma_band(k)

        # ================= PE =================
        if iA < NT:
            pq = ps.tile([P, 384], F32, name=f"pq_{iA}", tag="qkv", bufs=2,
                         padded_shape=[P, 512])
            for mj in range(8):
                nc.tensor.matmul(
                    pq[:],
                    lhsT=xts[:, mj, P * iA : P * (iA + 1)],
                    rhs=wqkv_sb[:, mj, :],
                    start=(mj == 0),
                    stop=(mj == 7),
                )
            qkv_ps[iA] = pq

        if 0 <= t1 < NT:
            bl = blocks_of(t1)
            qrhs = qkt_all[:, t1, 0 : 4 * P].rearrange("p (h q) -> p h q", h=HPC)
            sc_ps = ps.tile([P, 4, 512], F32, name=f"sc_{t1}", tag="sc", bufs=1)
            for kt, j in bl:
                nc.tensor.matmul(
                    sc_ps[:, j, :],
                    lhsT=qkt_all[:, kt, 4 * P : 4 * P + P],
                    rhs=qrhs,
                    start=True,
                    stop=True,
                )
            sc_ps_t[t1] = sc_ps

        if 0 <= iT < NT:
            rp = rp_t[iT]
            qkT = ps.tile([P, 3, P], BF16, name=f"qkT_{iT}", tag="qkv", bufs=2,
                          padded_shape=[P, 8, P])
            for hp in range(2):
                nc.tensor.transpose(
                    qkT[:, hp, :], rp[:, P * hp : P * (hp + 1)], ident_bf[:]
                )
            nc.tensor.transpose(qkT[0:64, 2, :], rp[:, 256:320], ident_bf[:])
            qkT_t[iT] = qkT

        if 0 <= t2 < NT:
            bl = blocks_of(t2)
            em = em_t[t2]
            j0e = blocks_of(t2)[0][1]
            pcx = ps.tile([P, 512], F32, name=f"pcx_{t2}", tag="cx", bufs=2)
            for bi, (kt, j) in enumerate(bl):
                nc.tensor.matmul(
                    pcx[0:65, :],
                    lhsT=v_all[:, kt, :],
                    rhs=em[:, j - j0e, :],
                    start=(bi == 0),
                    stop=(bi == len(bl) - 1),
                )
            pcx_t[t2] = pcx

        # ================= Act =================
        if iA < NT:
            pq = qkv_ps[iA]
            ssq = work.tile([P, G * DH], F32, tag="ssq")
            nc.scalar.square(ssq[:], pq[:, 0 : G * DH])
            ssq_t[iA] = ssq
            nc.scalar.copy(v_all[:, iA, 0:64], pq[:, 320:384])
        if 0 <= t1 < NT:
            bl = blocks_of(t1)
            j0, nb = bl[0][1], len(bl)
            ex = attn.tile([P, 4, 512], BF16, name=f"ex_{t1}", tag="ex")
            nc.scalar.activation(
                ex[:, j0 : j0 + nb, :], sc_ps_t[t1][:, j0 : j0 + nb, :],
                EXP, scale=SCALE,
            )
            ex_t[t1] = ex
            sc_ps_t[t1] = None
        if 0 <= t2 < NT:
            dn = denp.tile([1, 512], BF16, tag="dn")
            nc.scalar.copy(dn[:], pcx_t[t2][64:65, :])
            den_t[t2] = dn

        # ================= DVE (part 1) =================
        if 0 <= t3 < NT:
            dn = den_t[t3]
            g = g_sb[:, t3, :]
            nt_ = denp.tile([1, 512], BF16, tag="nt")
            nc.vector.tensor_tensor(nt_[:], dn[:], g, op=MULT)
            nu = denp.tile([1, 512], BF16, tag="nu")
            nc.vector.tensor_scalar(nu[:], nt_[:], -1.0, 2.0, op0=MULT, op1=ADD)
            ny1 = denp.tile([1, 512], BF16, tag="ny1")
            nc.vector.tensor_tensor(ny1[:], nu[:], g, op=MULT)
            nc.vector.tensor_tensor(nt_[:], dn[:], ny1[:], op=MULT)
            nc.vector.tensor_scalar(nu[:], nt_[:], -1.0, 2.0, op0=MULT, op1=ADD)
            rc = denp.tile([1, 512], F32, tag="rc")
            nc.vector.tensor_tensor(rc[:], nu[:], ny1[:], op=MULT)
            den_t[t3] = rc
        if iA < NT:
            pq = qkv_ps[iA]
            red = work.tile([P, G], F32, tag="red")
            nc.vector.tensor_reduce(
                red[:],
                ssq_t[iA][:].rearrange("p (g n) -> p g n", g=G),
                axis=mybir.AxisListType.X,
                op=ADD,
            )
            yu = work.tile([P, G], U32, tag="yu")
            nc.vector.tensor_scalar(
                yu[:], red[:].bitcast(U32), 1, None,
                op0=mybir.AluOpType.logical_shift_right,
            )
            y0 = work.tile([P, G], F32, tag="y0")
            nc.vector.tensor_tensor(
                y0[:].bitcast(U32),
                magic[:].bitcast(U32).broadcast_to([P, G]),
                yu[:],
                op=SUB,
            )
            ysq = work.tile([P, G], F32, tag="ysq")
            nc.vector.tensor_tensor(ysq[:], y0[:], y0[:], op=MULT)
            nc.vector.tensor_tensor(ysq[:], ysq[:], red[:], op=MULT)
            yw = work.tile([P, G], F32, tag="yw")
            nc.vector.tensor_scalar(yw[:], ysq[:], -0.5, 1.5, op0=MULT, op1=ADD)
            rsq = work.tile([P, G], F32, tag="rsq")
            nc.vector.tensor_tensor(rsq[:], y0[:], yw[:], op=MULT)
            qkn = work.tile([P, G * DH], BF16, tag="qkn")
            nc.vector.tensor_tensor(
                qkn[:].rearrange("p (g n) -> p g n", g=G),
                pq[:, 0 : G * DH].rearrange("p (g n) -> p g n", g=G),
                rsq[:].unsqueeze(-1).broadcast_to([P, G, DH]),
                op=MULT,
            )
            qv = qkn[:].rearrange("p (g n) -> p g n", g=G)
            cb = cos2_sb[:, iA, :].unsqueeze(1).broadcast_to([P, G, DH])
            sb = sin2_sb[:, iA, :].unsqueeze(1).broadcast_to([P, G, DH])
            ta = work.tile([P, G * DH], BF16, tag="ta")
            tb = work.tile([P, G * DH], BF16, tag="tb")
            tav = ta[:].rearrange("p (g n) -> p g n", g=G)
            tbv = tb[:].rearrange("p (g n) -> p g n", g=G)
            nc.vector.tensor_tensor(tav, qv, cb, op=MULT)
            nc.vector.tensor_tensor(tbv, qv, sb, op=MULT)
            rp = work.tile([P, G * DH], BF16, tag="rp")
            rv = rp[:].rearrange("p (g n) -> p g n", g=G)
            nc.vector.tensor_tensor(
                rv[:, :, 0:32], tav[:, :, 0:32], tbv[:, :, 32:64], op=SUB
            )
            nc.vector.tensor_tensor(
                rv[:, :, 32:64], tbv[:, :, 0:32], tav[:, :, 32:64], op=ADD
            )
            rp_t[iA] = rp

        # ================= Pool =================
        if 0 <= t3 < NT:
            rcb = attn.tile([64, 512], F32, name=f"rcb_{t3}", tag="rcb")
            nc.gpsimd.partition_broadcast(rcb[:], den_t[t3][:])
            rcb_t[t3] = rcb
        if 0 <= t1 < NT:
            bl = blocks_of(t1)
            j0, nb = bl[0][1], len(bl)
            em = attn.tile([P, 4, 512], BF16, name=f"em_{t1}", tag="em")
            mb = band_tiles[t1]
            nc.gpsimd.tensor_tensor(
                em[:, 0:nb, :].rearrange("p b (h q) -> p b h q", h=HPC),
                ex_t[t1][:, j0 : j0 + nb, :].rearrange(
                    "p b (h q) -> p b h q", h=HPC
                ),
                mb[:, j0 : j0 + nb, :].unsqueeze(2).broadcast_to([P, nb, HPC, P]),
                op=MULT,
            )
            em_t[t1] = em
            ex_t[t1] = None

        # ================= DVE (part 2) =================
        if 0 <= iT < NT:
            qkT = qkT_t[iT]
            qt_v = qkt_all[:, iT, 0 : 4 * P].rearrange("p (h q) -> p h q", h=HPC)
            nc.vector.tensor_copy(qt_v[:, 0::2, :], qkT[0:64, 0:2, :])
            nc.vector.tensor_copy(qt_v[:, 1::2, :], qkT[64:128, 0:2, :])
            nc.vector.tensor_copy(
                qkt_all[:, iT, 4 * P : 5 * P], qkT[0:64, 2, :]
            )
            qkT_t[iT] = None
        if 0 <= t3 < NT:
            pcx = pcx_t[t3]
            rcb = rcb_t[t3]
            sc_, qoff = t3 // 4, (t3 % 4) * P
            for half in range(2):
                nc.vector.tensor_tensor(
                    ctxt[sc_][64 * half : 64 * half + 64, :, qoff : qoff + P],
                    pcx[0:64, :].rearrange("p (h q) -> p h q", h=HPC)[
                        :, half::2, :
                    ],
                    rcb[:].rearrange("p (h q) -> p h q", h=HPC)[:, half::2, :],
                    op=MULT,
                )
            pcx_t[t3] = None
            rcb_t[t3] = None

    # ---------------- Phase C: output projection (transposed) ------------
    for sc in range(4):
        for mo in range(8):
            po = ps.tile([P, 512], F32, name=f"po_{sc}_{mo}", tag="qkv", bufs=2,
                         padded_shape=[P, 512])
            for c in range(2):
                nc.tensor.matmul(
                    po[:],
                    lhsT=wo_sb[:, c, P * mo : P * (mo + 1)],
                    rhs=ctxt[sc][:, c, :],
                    start=(c == 0),
                    stop=(c == 1),
                )
            ob = outp.tile([P, 512], BF16, tag="ob")
            if mo % 2 == 1:
                nc.scalar.copy(ob[:], po[:])
            else:
                nc.vector.tensor_copy(ob[:], po[:])
            nc.sync.dma_start(
                d["outT"][P * mo : P * (mo + 1), 512 * sc : 512 * (sc + 1)], ob[:]
            )


def build_program():
    nc = bacc.Bacc("TRN2", target_bir_lowering=False, debug=False, num_devices=8)
    d = {}
    d["xT"] = nc.dram_tensor("xT", [DM, S], BF16, kind="ExternalInput").ap()
    d["wqkv"] = nc.dram_tensor("wqkv", [DM, 384], BF16, kind="ExternalInput").ap()
    d["wo"] = nc.dram_tensor("wo", [256, DM], BF16, kind="ExternalInput").ap()
    d["cos2"] = nc.dram_tensor("cos2", [S, DH], BF16, kind="ExternalInput").ap()
    d["sin2"] = nc.dram_tensor("sin2", [S, DH], BF16, kind="ExternalInput").ap()
    d["band"] = nc.dram_tensor("band", [NT, P, 4, P], BF16, kind="ExternalInput").ap()
    d["gden"] = nc.dram_tensor("gden", [NT, 512], BF16, kind="ExternalInput").ap()
    d["outT"] = nc.dram_tensor("outT", [DM, S], BF16, kind="ExternalOutput").ap()
    with tile.TileContext(nc) as tc, ExitStack() as ctx:
        _build_kernel(ctx, tc, d)
    nc.compile()
    return nc


def make_masks(mask_np):
    """Band tiles [k,q]-oriented: blocks j=0..2 are k-tiles t-2+j, block 3 is
    the global block (k-tile 0, rows >= NGLOB zero, only used for t>=3)."""
    mask_np = np.asarray(mask_np).astype(bool)
    q = np.arange(S)[:, None]
    kk = np.arange(S)[None, :]
    wmask = ((kk <= q) & (kk > q - WINDOW)) | (kk < NGLOB)
    comb = mask_np[0, 0] & wmask  # [q, k]
    combT = comb.T.astype(np.float32)  # [k, q]
    band = np.zeros((NT, P, 4, P), np.float32)
    for t in range(NT):
        for kt in range(max(0, t - 2), t + 1):
            j = kt - (t - 2)
            band[t, :, j, :] = combT[P * kt : P * (kt + 1), P * t : P * (t + 1)]
        if t >= 3:
            band[t, 0:NGLOB, 3, :] = combT[0:NGLOB, P * t : P * (t + 1)]
    # 1/n(q) per (tile, h*q): the Newton seed for the softmax denominator
    n = comb.sum(axis=1).astype(np.float64)  # attended count per q row
    n = np.maximum(n, 1.0)
    gd = (1.0 / n).reshape(NT, P)
    gden = np.repeat(gd[:, None, :], HPC, axis=1).reshape(NT, 512)
    return band, gden


def make_in_maps(x, cos, sin, mask, Wq, Wk, Wv, Wo):
    import ml_dtypes

    bf = ml_dtypes.bfloat16
    x = np.asarray(x, np.float32)
    cos = np.asarray(cos, np.float32)
    sin = np.asarray(sin, np.float32)
    cos2 = np.concatenate([cos, cos], axis=1).astype(bf)
    sin2 = np.concatenate([sin, sin], axis=1).astype(bf)
    Wq, Wk, Wv = (np.asarray(a, np.float32) for a in (Wq, Wk, Wv))
    Wo = np.asarray(Wo, np.float32).astype(bf)
    band, gden = make_masks(mask)
    band = band.astype(bf)
    gden = gden.astype(bf)
    xT = [np.ascontiguousarray(x[b].T).astype(bf) for b in range(B)]
    in_maps = []
    for c in range(8):
        b, g = divmod(c, 4)
        wqkv = np.concatenate(
            [
                Wq[:, 256 * g : 256 * (g + 1)],
                Wk[:, 64 * g : 64 * (g + 1)],
                Wv[:, 64 * g : 64 * (g + 1)],
            ],
            axis=1,
        ).astype(bf)
        in_maps.append(
            {
                "xT": xT[b],
                "wqkv": np.ascontiguousarray(wqkv),
                "wo": np.ascontiguousarray(Wo[256 * g : 256 * (g + 1), :]),
                "cos2": cos2,
                "sin2": sin2,
                "band": band,
                "gden": gden,
            }
        )
    return in_maps


_PROGRAM = None


def _get_program():
    global _PROGRAM
    if _PROGRAM is None:
        _PROGRAM = build_program()
    return _PROGRAM


def kernel(x, cos, sin, mask, Wq, Wk, Wv, Wo, _trace=False, _trace_kwargs=None):
    nc = _get_program()
    in_maps = make_in_maps(x, cos, sin, mask, Wq, Wk, Wv, Wo)
    res = run_bass_kernel_spmd(
        nc, in_maps, list(range(8)), trace=_trace, **(_trace_kwargs or {})
    )
    out = np.zeros((B, S, DM), np.float32)
    for c in range(8):
        out[c // 4] += res.results[c]["outT"].T.astype(np.float32)
    if _trace:
        kernel._last_results = res
    return out


# revision 19
# speedup vs baseline: 2.2282x; 1.8919x over previous
"""Trainium2 Bass kernel for GroupedQueryAttention (sliding-window + global).

Sharding: 8 cores = 2 (batch) x 4 (GQA groups). Core c handles batch c//4 and
kv-head g=c%4 with its 4 query heads. Wq/Wk/Wv column-sharded, Wo row-sharded;
each core emits outT = (context_g @ Wo_g)^T in bf16; the host transposes,
upcasts and sums partials per batch.

v3 design notes:
- Host pre-transposes and pre-casts x to bf16 (xT): no device-side x
  transposes or f32->bf16 casts.
- Scalar engine uses only {Exp, Square, Copy} (all live in the exp
  activation table): zero ACT_TABLE_LOAD swaps. The L2-norm rsqrt and the
  softmax-denominator reciprocal run as DVE Newton iterations (bit-trick
  seed for rsqrt; host-provided 1/attended-count seed for the denominator,
  avoiding the ~3.3us hardware RECIPROCAL).
- Deep software pipeline: iter k = QKV(k) | transposes(k-1) |
  scores+exp+mask(k-2) | ctx(k-3) | recip+divide(k-4), giving every
  cross-engine dependency >= 1 iteration of slack so the tensor engine
  streams continuously and ramps to its fast p-state.
- PSUM budget: qkv tag 2 banks + one 4-bank score tile + ctx 2 banks = 8.
"""

import sys

for _p in (
    "/opt/trn_rl_repo",
    "/root/.axon_site",
    "/root/.axon_site/_ro/pypackages",
    "/root/.axon_site/_ro/trn_rl_repo",
):
    if _p not in sys.path:
        sys.path.insert(0, _p)

from contextlib import ExitStack

import numpy as np

import concourse.bass as bass  # noqa: F401  (registers engine classes)
import concourse.tile as tile
from concourse import bacc, mybir
from concourse.bass_utils import run_bass_kernel_spmd
from concourse.masks import make_identity

B, S, DM = 2, 2048, 1024
NH, NKV, DH = 16, 4, 64
HPC = 4
WINDOW, NGLOB = 256, 4
SCALE = 1.0 / np.sqrt(DH)
CAP = 15.0
EPS = 1e-8
P = 128
NT = S // P
G = HPC + 1
F32 = mybir.dt.float32
U32 = mybir.dt.uint32
BF16 = mybir.dt.bfloat16
MULT = mybir.AluOpType.mult
ADD = mybir.AluOpType.add
SUB = mybir.AluOpType.subtract
EXP = mybir.ActivationFunctionType.Exp
RSQRT_MAGIC = float(np.frombuffer(np.uint32(0x5F3759DF).tobytes(), np.float32)[0])


def _build_kernel(ctx, tc, d):
    nc = tc.nc

    consts = ctx.enter_context(tc.tile_pool(name="consts", bufs=1))
    ident_bf = consts.tile([P, P], BF16)
    identf = consts.tile([P, P], F32)
    make_identity(nc, identf[:])
    nc.vector.tensor_copy(ident_bf[:], identf[:])
    magic = consts.tile([P, 1], F32)
    nc.vector.memset(magic[:], RSQRT_MAGIC)

    wqkv_sb = consts.tile([P, 8, 384], BF16)
    wo_sb = consts.tile([P, 2, DM], BF16)
    cos2_sb = consts.tile([P, NT, DH], BF16)
    sin2_sb = consts.tile([P, NT, DH], BF16)
    g_sb = consts.tile([1, NT, 512], BF16)

    xts = consts.tile([P, 8, S], BF16)
    qkt_all = consts.tile([64, NT, 5 * P], BF16)  # 4 q heads then k, per tile
    v_all = consts.tile([P, NT, 65], BF16)
    ctxt = [consts.tile([P, 2, 512], BF16, name=f"ctxt_{sc}") for sc in range(4)]

    for i in range(NT):
        nc.gpsimd.memset(v_all[:, i, 64:65], 1.0)

    work = ctx.enter_context(tc.tile_pool(name="work", bufs=3))
    attn = ctx.enter_context(tc.tile_pool(name="attn", bufs=3))
    denp = ctx.enter_context(tc.tile_pool(name="denp", bufs=3))
    outp = ctx.enter_context(tc.tile_pool(name="outp", bufs=4))
    mbp = ctx.enter_context(tc.tile_pool(name="mbp", bufs=4))

    ps = ctx.enter_context(tc.tile_pool(name="ps", bufs=1, space="PSUM"))

    def dma_x(i):
        if 0 <= i < NT:
            nc.sync.dma_start(
                xts[:, :, P * i : P * (i + 1)],
                d["xT"].rearrange("(c p) s -> p c s", p=P)[:, :, P * i : P * (i + 1)],
            )

    band_tiles = [None] * NT

    def dma_band(t):
        if 0 <= t < NT:
            mb = mbp.tile([P, 4, P], BF16, name=f"mb_{t}", tag="mb")
            nc.sync.dma_start(mb[:], d["band"][t])
            band_tiles[t] = mb

    def blocks_of(t):
        bl = [(kt, kt - (t - 2)) for kt in range(max(0, t - 2), t + 1)]
        if t >= 3:
            bl.append((0, 3))
        return bl

    qkv_ps = [None] * NT
    ssq_t = [None] * NT
    rp_t = [None] * NT
    qkT_t = [None] * NT
    ex_t = [None] * NT
    em_t = [None] * NT
    sc_ps_t = [None] * NT
    pcx_t = [None] * NT
    den_t = [None] * NT
    dn_t = [None] * NT
    rc_neg_t = [None] * NT
    rcb_t = [None] * NT

    dma_x(0)
    nc.sync.dma_start(wqkv_sb[:], d["wqkv"].rearrange("(c p) n -> p c n", p=P))
    nc.sync.dma_start(cos2_sb[:], d["cos2"].rearrange("(t p) n -> p t n", p=P))
    nc.sync.dma_start(sin2_sb[:], d["sin2"].rearrange("(t p) n -> p t n", p=P))
    dma_x(1)

    for k in range(NT + 4):
        iA = k       # QKV + norm + rope
        iT = k - 1   # transposes + qkT copies
        t1 = k - 2   # scores + exp + mask
        t2 = k - 3   # ctx matmuls + den extract
        t3 = k - 4   # reciprocal + divide

        dma_x(iA + 2)
        dma_band(k)
        if k == 1:
            nc.sync.dma_start(g_sb[:], d["gden"].unsqueeze(0))
        if k == 8:
            nc.sync.dma_start(wo_sb[:], d["wo"].rearrange("(c p) n -> p c n", p=P))

        # ================= PE =================
        if iA < NT:
            pq = ps.tile([P, 384], F32, name=f"pq_{iA}", tag="qkv", bufs=2,
                         padded_shape=[P, 512])
            for mj in range(8):
                nc.tensor.matmul(
                    pq[:],
                    lhsT=xts[:, mj, P * iA : P * (iA + 1)],
                    rhs=wqkv_sb[:, mj, :],
                    start=(mj == 0),
                    stop=(mj == 7),
                )
            qkv_ps[iA] = pq

        if 0 <= t1 < NT:
            bl = blocks_of(t1)
            qrhs = qkt_all[:, t1, 0 : 4 * P].rearrange("p (h q) -> p h q", h=HPC)
            sc_ps = ps.tile([P, 4, 512], F32, name=f"sc_{t1}", tag="sc", bufs=1)
            for kt, j in bl:
                nc.tensor.matmul(
                    sc_ps[:, j, :],
                    lhsT=qkt_all[:, kt, 4 * P : 4 * P + P],
                    rhs=qrhs,
                    start=True,
                    stop=True,
                )
            sc_ps_t[t1] = sc_ps

        if 0 <= iT < NT:
            rp = rp_t[iT]
            qkT = ps.tile([P, 3, P], BF16, name=f"qkT_{iT}", tag="qkv", bufs=2,
                          padded_shape=[P, 8, P])
            for hp in range(2):
                nc.tensor.transpose(
                    qkT[:, hp, :], rp[:, P * hp : P * (hp + 1)], ident_bf[:]
                )
            nc.tensor.transpose(qkT[0:64, 2, :], rp[:, 256:320], ident_bf[:])
            qkT_t[iT] = qkT

        if 0 <= t2 < NT:
            bl = blocks_of(t2)
            em = em_t[t2]
            j0e = blocks_of(t2)[0][1]
            pcx = ps.tile([P, 512], F32, name=f"pcx_{t2}", tag="cx", bufs=2)
            for bi, (kt, j) in enumerate(bl):
                nc.tensor.matmul(
                    pcx[0:65, :],
                    lhsT=v_all[:, kt, :],
                    rhs=em[:, j - j0e, :],
                    start=(bi == 0),
                    stop=(bi == len(bl) - 1),
                )
            pcx_t[t2] = pcx

        # ================= Act =================
        if iA < NT:
            pq = qkv_ps[iA]
            ssq = work.tile([P, G * DH], F32, tag="ssq")
            nc.scalar.square(ssq[:], pq[:, 0 : G * DH])
            ssq_t[iA] = ssq
            nc.scalar.copy(v_all[:, iA, 0:64], pq[:, 320:384])
        if 0 <= t1 < NT:
            bl = blocks_of(t1)
            j0, nb = bl[0][1], len(bl)
            ex = attn.tile([P, 4, 512], BF16, name=f"ex_{t1}", tag="ex")
            nc.scalar.activation(
                ex[:, j0 : j0 + nb, :], sc_ps_t[t1][:, j0 : j0 + nb, :],
                EXP, scale=SCALE,
            )
            ex_t[t1] = ex
            sc_ps_t[t1] = None
        if 0 <= t2 < NT:
            ub = attn.tile([64, 512], BF16, name=f"ub_{t2}", tag="ub")
            nc.scalar.copy(ub[:], pcx_t[t2][0:64, :])
            den_t[t2] = ub
            dnc = denp.tile([1, 512], BF16, tag="dnc")
            nc.scalar.copy(dnc[:], pcx_t[t2][64:65, :])
            dn_t[t2] = dnc
            pcx_t[t2] = None

        # ================= DVE (part 1) =================
        if 0 <= t3 < NT:
            dn = dn_t[t3][:]  # +den row (bf16, base partition 0)
            g = g_sb[:, t3, :]
            nt_ = denp.tile([1, 512], BF16, tag="nt")
            nc.vector.tensor_tensor(nt_[:], dn, g, op=MULT)
            ny1 = denp.tile([1, 512], BF16, tag="ny1")
            nc.vector.scalar_tensor_tensor(ny1[:], nt_[:], 2.0, g, op0=SUB, op1=MULT)
            nc.vector.tensor_tensor(nt_[:], dn, ny1[:], op=MULT)
            rc = denp.tile([1, 512], BF16, tag="rc")
            nc.vector.scalar_tensor_tensor(
                rc[:], nt_[:], -2.0, ny1[:], op0=SUB, op1=MULT
            )
            rc_neg_t[t3] = rc
        if iA < NT:
            pq = qkv_ps[iA]
            red = work.tile([P, G], F32, tag="red")
            nc.vector.tensor_reduce(
                red[:],
                ssq_t[iA][:].rearrange("p (g n) -> p g n", g=G),
                axis=mybir.AxisListType.X,
                op=ADD,
            )
            yu = work.tile([P, G], U32, tag="yu")
            nc.vector.tensor_scalar(
                yu[:], red[:].bitcast(U32), 1, None,
                op0=mybir.AluOpType.logical_shift_right,
            )
            y0 = work.tile([P, G], F32, tag="y0")
            nc.vector.tensor_tensor(
                y0[:].bitcast(U32),
                magic[:].bitcast(U32).broadcast_to([P, G]),
                yu[:],
                op=SUB,
            )
            ysq = work.tile([P, G], F32, tag="ysq")
            nc.vector.tensor_tensor(ysq[:], y0[:], y0[:], op=MULT)
            nc.vector.tensor_tensor(ysq[:], ysq[:], red[:], op=MULT)
            rsq = work.tile([P, G], F32, tag="rsq")
            nc.vector.scalar_tensor_tensor(
                rsq[:], ysq[:], 3.0, y0[:], op0=SUB, op1=MULT
            )
            nc.vector.tensor_scalar(rsq[:], rsq[:], -0.5, None, op0=MULT)
            qkn = work.tile([P, G * DH], BF16, tag="qkn")
            nc.vector.tensor_tensor(
                qkn[:].rearrange("p (g n) -> p g n", g=G),
                pq[:, 0 : G * DH].rearrange("p (g n) -> p g n", g=G),
                rsq[:].unsqueeze(-1).broadcast_to([P, G, DH]),
                op=MULT,
            )
            qv = qkn[:].rearrange("p (g n) -> p g n", g=G)
            cb = cos2_sb[:, iA, :].unsqueeze(1).broadcast_to([P, G, DH])
            sb = sin2_sb[:, iA, :].unsqueeze(1).broadcast_to([P, G, DH])
            ta = work.tile([P, G * DH], BF16, tag="ta")
            tb = work.tile([P, G * DH], BF16, tag="tb")
            tav = ta[:].rearrange("p (g n) -> p g n", g=G)
            tbv = tb[:].rearrange("p (g n) -> p g n", g=G)
            nc.vector.tensor_tensor(tav, qv, cb, op=MULT)
            nc.vector.tensor_tensor(tbv, qv, sb, op=MULT)
            rp = work.tile([P, G * DH], BF16, tag="rp")
            rv = rp[:].rearrange("p (g n) -> p g n", g=G)
            nc.vector.tensor_tensor(
                rv[:, :, 0:32], tav[:, :, 0:32], tbv[:, :, 32:64], op=SUB
            )
            nc.vector.tensor_tensor(
                rv[:, :, 32:64], tbv[:, :, 0:32], tav[:, :, 32:64], op=ADD
            )
            rp_t[iA] = rp

        # ================= Pool =================
        if 0 <= t3 < NT:
            rcb = attn.tile([64, 512], BF16, name=f"rcb_{t3}", tag="rcb")
            nc.gpsimd.partition_broadcast(rcb[:], rc_neg_t[t3][:])
            rcb_t[t3] = rcb
        if 0 <= t1 < NT:
            bl = blocks_of(t1)
            j0, nb = bl[0][1], len(bl)
            em = attn.tile([P, 4, 512], BF16, name=f"em_{t1}", tag="em")
            mb = band_tiles[t1]
            nc.vector.tensor_tensor(
                em[:, 0:nb, :].rearrange("p b (h q) -> p b h q", h=HPC),
                ex_t[t1][:, j0 : j0 + nb, :].rearrange(
                    "p b (h q) -> p b h q", h=HPC
                ),
                mb[:, j0 : j0 + nb, :].unsqueeze(2).broadcast_to([P, nb, HPC, P]),
                op=MULT,
            )
            em_t[t1] = em
            ex_t[t1] = None

        # ================= Act (tail): qkT copies =================
        if 0 <= iT < NT:
            qkT = qkT_t[iT]
            qt_v = qkt_all[:, iT, 0 : 4 * P].rearrange("p (h q) -> p h q", h=HPC)
            nc.scalar.copy(qt_v[:, 0::2, :], qkT[0:64, 0:2, :])
            nc.scalar.copy(qt_v[:, 1::2, :], qkT[64:128, 0:2, :])
            nc.scalar.copy(qkt_all[:, iT, 4 * P : 5 * P], qkT[0:64, 2, :])
            qkT_t[iT] = None
        if 0 <= t3 < NT:
            ub = den_t[t3]
            rcb = rcb_t[t3]
            sc_, qoff = t3 // 4, (t3 % 4) * P
            for half in range(2):
                nc.vector.scalar_tensor_tensor(
                    ctxt[sc_][64 * half : 64 * half + 64, :, qoff : qoff + P],
                    ub[:].rearrange("p (h q) -> p h q", h=HPC)[
                        :, half::2, :
                    ],
                    -1.0,
                    rcb[:].rearrange("p (h q) -> p h q", h=HPC)[:, half::2, :],
                    op0=MULT,
                    op1=MULT,
                )
            den_t[t3] = None
            rcb_t[t3] = None

    # ---------------- Phase C: output projection (transposed) ------------
    for sc in range(4):
        for mo in range(8):
            po = ps.tile([P, 512], F32, name=f"po_{sc}_{mo}", tag="qkv", bufs=2,
                         padded_shape=[P, 512])
            for c in range(2):
                nc.tensor.matmul(
                    po[:],
                    lhsT=wo_sb[:, c, P * mo : P * (mo + 1)],
                    rhs=ctxt[sc][:, c, :],
                    start=(c == 0),
                    stop=(c == 1),
                )
            ob = outp.tile([P, 512], BF16, tag="ob")
            nc.scalar.copy(ob[:], po[:])
            nc.sync.dma_start(
                d["outT"][P * mo : P * (mo + 1), 512 * sc : 512 * (sc + 1)], ob[:]
            )


def build_program():
    nc = bacc.Bacc("TRN2", target_bir_lowering=False, debug=False, num_devices=8)
    d = {}
    d["xT"] = nc.dram_tensor("xT", [DM, S], BF16, kind="ExternalInput").ap()
    d["wqkv"] = nc.dram_tensor("wqkv", [DM, 384], BF16, kind="ExternalInput").ap()
    d["wo"] = nc.dram_tensor("wo", [256, DM], BF16, kind="ExternalInput").ap()
    d["cos2"] = nc.dram_tensor("cos2", [S, DH], BF16, kind="ExternalInput").ap()
    d["sin2"] = nc.dram_tensor("sin2", [S, DH], BF16, kind="ExternalInput").ap()
    d["band"] = nc.dram_tensor("band", [NT, P, 4, P], BF16, kind="ExternalInput").ap()
    d["gden"] = nc.dram_tensor("gden", [NT, 512], BF16, kind="ExternalInput").ap()
    d["outT"] = nc.dram_tensor("outT", [DM, S], BF16, kind="ExternalOutput").ap()
    with tile.TileContext(nc) as tc, ExitStack() as ctx:
        _build_kernel(ctx, tc, d)
    nc.compile()
    return nc


def make_masks(mask_np):
    """Band tiles [k,q]-oriented: blocks j=0..2 are k-tiles t-2+j, block 3 is
    the global block (k-tile 0, rows >= NGLOB zero, only used for t>=3)."""
    mask_np = np.asarray(mask_np).astype(bool)
    q = np.arange(S)[:, None]
    kk = np.arange(S)[None, :]
    wmask = ((kk <= q) & (kk > q - WINDOW)) | (kk < NGLOB)
    comb = mask_np[0, 0] & wmask  # [q, k]
    combT = comb.T.astype(np.float32)  # [k, q]
    band = np.zeros((NT, P, 4, P), np.float32)
    for t in range(NT):
        for kt in range(max(0, t - 2), t + 1):
            j = kt - (t - 2)
            band[t, :, j, :] = combT[P * kt : P * (kt + 1), P * t : P * (t + 1)]
        if t >= 3:
            band[t, 0:NGLOB, 3, :] = combT[0:NGLOB, P * t : P * (t + 1)]
    # 1/n(q) per (tile, h*q): the Newton seed for the softmax denominator
    n = comb.sum(axis=1).astype(np.float64)  # attended count per q row
    n = np.maximum(n, 1.0)
    gd = (1.0 / n).reshape(NT, P)
    gden = np.repeat(gd[:, None, :], HPC, axis=1).reshape(NT, 512)
    return band, gden


def make_in_maps(x, cos, sin, mask, Wq, Wk, Wv, Wo):
    import ml_dtypes

    bf = ml_dtypes.bfloat16
    x = np.asarray(x, np.float32)
    cos = np.asarray(cos, np.float32)
    sin = np.asarray(sin, np.float32)
    cos2 = np.concatenate([cos, cos], axis=1).astype(bf)
    sin2 = np.concatenate([sin, sin], axis=1).astype(bf)
    Wq, Wk, Wv = (np.asarray(a, np.float32) for a in (Wq, Wk, Wv))
    Wo = np.asarray(Wo, np.float32).astype(bf)
    band, gden = make_masks(mask)
    band = band.astype(bf)
    gden = gden.astype(bf)
    xT = [np.ascontiguousarray(x[b].T).astype(bf) for b in range(B)]
    in_maps = []
    for c in range(8):
        b, g = divmod(c, 4)
        wqkv = np.concatenate(
            [
                Wq[:, 256 * g : 256 * (g + 1)],
                Wk[:, 64 * g : 64 * (g + 1)],
                Wv[:, 64 * g : 64 * (g + 1)],
            ],
            axis=1,
        ).astype(bf)
        in_maps.append(
            {
                "xT": xT[b],
                "wqkv": np.ascontiguousarray(wqkv),
                "wo": np.ascontiguousarray(Wo[256 * g : 256 * (g + 1), :]),
                "cos2": cos2,
                "sin2": sin2,
                "band": band,
                "gden": gden,
            }
        )
    return in_maps


_PROGRAM = None


def _get_program():
    global _PROGRAM
    if _PROGRAM is None:
        _PROGRAM = build_program()
    return _PROGRAM


def kernel(x, cos, sin, mask, Wq, Wk, Wv, Wo, _trace=False, _trace_kwargs=None):
    nc = _get_program()
    in_maps = make_in_maps(x, cos, sin, mask, Wq, Wk, Wv, Wo)
    res = run_bass_kernel_spmd(
        nc, in_maps, list(range(8)), trace=_trace, **(_trace_kwargs or {})
    )
    out = np.zeros((B, S, DM), np.float32)
    for c in range(8):
        out[c // 4] += res.results[c]["outT"].T.astype(np.float32)
    if _trace:
        kernel._last_results = res
    return out


# revision 20
# speedup vs baseline: 2.2974x; 1.0310x over previous
"""Trainium2 Bass kernel for GroupedQueryAttention (sliding-window + global).

Sharding: 8 cores = 2 (batch) x 4 (GQA groups). Core c handles batch c//4 and
kv-head g=c%4 with its 4 query heads. Wq/Wk/Wv column-sharded, Wo row-sharded;
each core emits outT = (context_g @ Wo_g)^T in bf16; the host transposes,
upcasts and sums partials per batch.

v3 design notes:
- Host pre-transposes and pre-casts x to bf16 (xT): no device-side x
  transposes or f32->bf16 casts.
- Scalar engine uses only {Exp, Square, Copy} (all live in the exp
  activation table): zero ACT_TABLE_LOAD swaps. The L2-norm rsqrt and the
  softmax-denominator reciprocal run as DVE Newton iterations (bit-trick
  seed for rsqrt; host-provided 1/attended-count seed for the denominator,
  avoiding the ~3.3us hardware RECIPROCAL).
- Deep software pipeline: iter k = QKV(k) | transposes(k-1) |
  scores+exp+mask(k-2) | ctx(k-3) | recip+divide(k-4), giving every
  cross-engine dependency >= 1 iteration of slack so the tensor engine
  streams continuously and ramps to its fast p-state.
- PSUM budget: qkv tag 2 banks + one 4-bank score tile + ctx 2 banks = 8.
"""

import sys

for _p in (
    "/opt/trn_rl_repo",
    "/root/.axon_site",
    "/root/.axon_site/_ro/pypackages",
    "/root/.axon_site/_ro/trn_rl_repo",
):
    if _p not in sys.path:
        sys.path.insert(0, _p)

from contextlib import ExitStack

import numpy as np

import concourse.bass as bass  # noqa: F401  (registers engine classes)
import concourse.tile as tile
from concourse import bacc, mybir
from concourse.bass_utils import run_bass_kernel_spmd
from concourse.masks import make_identity

B, S, DM = 2, 2048, 1024
NH, NKV, DH = 16, 4, 64
HPC = 4
WINDOW, NGLOB = 256, 4
SCALE = 1.0 / np.sqrt(DH)
CAP = 15.0
EPS = 1e-8
P = 128
NT = S // P
G = HPC + 1
F32 = mybir.dt.float32
U32 = mybir.dt.uint32
BF16 = mybir.dt.bfloat16
MULT = mybir.AluOpType.mult
ADD = mybir.AluOpType.add
SUB = mybir.AluOpType.subtract
EXP = mybir.ActivationFunctionType.Exp
RSQRT_MAGIC = float(np.frombuffer(np.uint32(0x5F3759DF).tobytes(), np.float32)[0])


def _build_kernel(ctx, tc, d):
    nc = tc.nc

    consts = ctx.enter_context(tc.tile_pool(name="consts", bufs=1))
    ident_bf = consts.tile([P, P], BF16)
    identf = consts.tile([P, P], F32)
    make_identity(nc, identf[:])
    nc.vector.tensor_copy(ident_bf[:], identf[:])
    magic = consts.tile([P, 1], F32)
    nc.vector.memset(magic[:], RSQRT_MAGIC)

    wqkv_sb = consts.tile([P, 8, 384], BF16)
    wo_sb = consts.tile([P, 2, DM], BF16)
    cos2_sb = consts.tile([P, NT, DH], BF16)
    sin2_sb = consts.tile([P, NT, DH], BF16)
    g_sb = consts.tile([1, NT, 512], BF16)

    xts = consts.tile([P, 8, S], BF16)
    qkt_all = consts.tile([64, NT, 5 * P], BF16)  # 4 q heads then k, per tile
    v_all = consts.tile([P, NT, 65], BF16)
    ctxt = [consts.tile([P, 2, 512], BF16, name=f"ctxt_{sc}") for sc in range(4)]

    for i in range(NT):
        nc.gpsimd.memset(v_all[:, i, 64:65], 1.0)

    work = ctx.enter_context(tc.tile_pool(name="work", bufs=3))
    attn = ctx.enter_context(tc.tile_pool(name="attn", bufs=3))
    denp = ctx.enter_context(tc.tile_pool(name="denp", bufs=3))
    outp = ctx.enter_context(tc.tile_pool(name="outp", bufs=4))
    mbp = ctx.enter_context(tc.tile_pool(name="mbp", bufs=4))

    ps = ctx.enter_context(tc.tile_pool(name="ps", bufs=1, space="PSUM"))

    def dma_x(i):
        if 0 <= i < NT:
            nc.sync.dma_start(
                xts[:, :, P * i : P * (i + 1)],
                d["xT"].rearrange("(c p) s -> p c s", p=P)[:, :, P * i : P * (i + 1)],
            )

    band_tiles = [None] * NT

    def dma_band(t):
        if 0 <= t < NT:
            mb = mbp.tile([P, 4, P], BF16, name=f"mb_{t}", tag="mb")
            nc.sync.dma_start(mb[:], d["band"][t])
            band_tiles[t] = mb

    def blocks_of(t):
        bl = [(kt, kt - (t - 2)) for kt in range(max(0, t - 2), t + 1)]
        if t >= 3:
            bl.append((0, 3))
        return bl

    qkv_ps = [None] * NT
    ssq_t = [None] * NT
    rp_t = [None] * NT
    qkT_t = [None] * NT
    ex_t = [None] * NT
    em_t = [None] * NT
    sc_ps_t = [None] * NT
    pcx_t = [None] * NT
    den_t = [None] * NT
    dn_t = [None] * NT
    rc_neg_t = [None] * NT
    rcb_t = [None] * NT

    def emit_C(sc):
        for mo in range(8):
            po = ps.tile([P, 512], F32, name=f"po_{sc}_{mo}", tag="qkv", bufs=2,
                         padded_shape=[P, 512])
            for c in range(2):
                nc.tensor.matmul(
                    po[:],
                    lhsT=wo_sb[:, c, P * mo : P * (mo + 1)],
                    rhs=ctxt[sc][:, c, :],
                    start=(c == 0),
                    stop=(c == 1),
                )
            ob = outp.tile([P, 512], BF16, tag="ob")
            if mo % 2 == 1:
                nc.scalar.copy(ob[:], po[:])
            else:
                nc.vector.tensor_copy(ob[:], po[:])
            nc.sync.dma_start(
                d["outT"][P * mo : P * (mo + 1), 512 * sc : 512 * (sc + 1)], ob[:]
            )

    dma_x(0)
    nc.sync.dma_start(wqkv_sb[:], d["wqkv"].rearrange("(c p) n -> p c n", p=P))
    nc.sync.dma_start(cos2_sb[:], d["cos2"].rearrange("(t p) n -> p t n", p=P))
    nc.sync.dma_start(sin2_sb[:], d["sin2"].rearrange("(t p) n -> p t n", p=P))
    dma_x(1)

    for k in range(NT + 4):
        iA = k       # QKV + norm + rope
        iT = k - 1   # transposes + qkT copies
        t1 = k - 2   # scores + exp + mask
        t2 = k - 3   # ctx matmuls + den extract
        t3 = k - 4   # reciprocal + divide

        dma_x(iA + 2)
        dma_band(k)
        if k == 1:
            nc.sync.dma_start(g_sb[:], d["gden"].unsqueeze(0))
        if k == 8:
            nc.sync.dma_start(wo_sb[:], d["wo"].rearrange("(c p) n -> p c n", p=P))

        # ================= PE =================
        if iA < NT:
            pq = ps.tile([P, 384], F32, name=f"pq_{iA}", tag="qkv", bufs=2,
                         padded_shape=[P, 512])
            for mj in range(8):
                nc.tensor.matmul(
                    pq[:],
                    lhsT=xts[:, mj, P * iA : P * (iA + 1)],
                    rhs=wqkv_sb[:, mj, :],
                    start=(mj == 0),
                    stop=(mj == 7),
                )
            qkv_ps[iA] = pq

        if 0 <= t1 < NT:
            bl = blocks_of(t1)
            qrhs = qkt_all[:, t1, 0 : 4 * P].rearrange("p (h q) -> p h q", h=HPC)
            sc_ps = ps.tile([P, 4, 512], F32, name=f"sc_{t1}", tag="sc", bufs=1)
            for kt, j in bl:
                nc.tensor.matmul(
                    sc_ps[:, j, :],
                    lhsT=qkt_all[:, kt, 4 * P : 4 * P + P],
                    rhs=qrhs,
                    start=True,
                    stop=True,
                )
            sc_ps_t[t1] = sc_ps

        if 0 <= iT < NT:
            rp = rp_t[iT]
            qkT = ps.tile([P, 3, P], BF16, name=f"qkT_{iT}", tag="qkv", bufs=2,
                          padded_shape=[P, 8, P])
            for hp in range(2):
                nc.tensor.transpose(
                    qkT[:, hp, :], rp[:, P * hp : P * (hp + 1)], ident_bf[:]
                )
            nc.tensor.transpose(qkT[0:64, 2, :], rp[:, 256:320], ident_bf[:])
            qkT_t[iT] = qkT

        if 0 <= t2 < NT:
            bl = blocks_of(t2)
            em = em_t[t2]
            j0e = blocks_of(t2)[0][1]
            pcx = ps.tile([P, 512], F32, name=f"pcx_{t2}", tag="cx", bufs=2)
            for bi, (kt, j) in enumerate(bl):
                nc.tensor.matmul(
                    pcx[0:65, :],
                    lhsT=v_all[:, kt, :],
                    rhs=em[:, j - j0e, :],
                    start=(bi == 0),
                    stop=(bi == len(bl) - 1),
                )
            pcx_t[t2] = pcx

        if 16 <= k <= 18:
            emit_C(k - 16)

        # ================= Act =================
        if iA < NT:
            pq = qkv_ps[iA]
            ssq = work.tile([P, G * DH], F32, tag="ssq")
            nc.scalar.square(ssq[:], pq[:, 0 : G * DH])
            ssq_t[iA] = ssq
            nc.scalar.copy(v_all[:, iA, 0:64], pq[:, 320:384])
        if 0 <= t1 < NT:
            bl = blocks_of(t1)
            j0, nb = bl[0][1], len(bl)
            ex = attn.tile([P, 4, 512], BF16, name=f"ex_{t1}", tag="ex")
            nc.scalar.activation(
                ex[:, j0 : j0 + nb, :], sc_ps_t[t1][:, j0 : j0 + nb, :],
                EXP, scale=SCALE,
            )
            ex_t[t1] = ex
            sc_ps_t[t1] = None
        if 0 <= t2 < NT:
            ub = attn.tile([64, 512], BF16, name=f"ub_{t2}", tag="ub")
            nc.scalar.copy(ub[:], pcx_t[t2][0:64, :])
            den_t[t2] = ub
            dnc = denp.tile([1, 512], BF16, tag="dnc")
            nc.scalar.copy(dnc[:], pcx_t[t2][64:65, :])
            dn_t[t2] = dnc
            pcx_t[t2] = None

        # ================= DVE (part 1) =================
        if 0 <= t3 < NT:
            dn = dn_t[t3][:]  # +den row (bf16, base partition 0)
            g = g_sb[:, t3, :]
            nt_ = denp.tile([1, 512], BF16, tag="nt")
            nc.vector.tensor_tensor(nt_[:], dn, g, op=MULT)
            ny1 = denp.tile([1, 512], BF16, tag="ny1")
            nc.vector.scalar_tensor_tensor(ny1[:], nt_[:], 2.0, g, op0=SUB, op1=MULT)
            nc.vector.tensor_tensor(nt_[:], dn, ny1[:], op=MULT)
            rc = denp.tile([1, 512], BF16, tag="rc")
            nc.vector.scalar_tensor_tensor(
                rc[:], nt_[:], -2.0, ny1[:], op0=SUB, op1=MULT
            )
            rc_neg_t[t3] = rc
        if iA < NT:
            pq = qkv_ps[iA]
            red = work.tile([P, G], F32, tag="red")
            nc.vector.tensor_reduce(
                red[:],
                ssq_t[iA][:].rearrange("p (g n) -> p g n", g=G),
                axis=mybir.AxisListType.X,
                op=ADD,
            )
            yu = work.tile([P, G], U32, tag="yu")
            nc.vector.tensor_scalar(
                yu[:], red[:].bitcast(U32), 1, None,
                op0=mybir.AluOpType.logical_shift_right,
            )
            y0 = work.tile([P, G], F32, tag="y0")
            nc.vector.tensor_tensor(
                y0[:].bitcast(U32),
                magic[:].bitcast(U32).broadcast_to([P, G]),
                yu[:],
                op=SUB,
            )
            ysq = work.tile([P, G], F32, tag="ysq")
            nc.vector.tensor_tensor(ysq[:], y0[:], y0[:], op=MULT)
            nc.vector.tensor_tensor(ysq[:], ysq[:], red[:], op=MULT)
            rsq = work.tile([P, G], F32, tag="rsq")
            nc.vector.scalar_tensor_tensor(
                rsq[:], ysq[:], 3.0, y0[:], op0=SUB, op1=MULT
            )
            nc.vector.tensor_scalar(rsq[:], rsq[:], -0.5, None, op0=MULT)
            qkn = work.tile([P, G * DH], BF16, tag="qkn")
            nc.vector.tensor_tensor(
                qkn[:].rearrange("p (g n) -> p g n", g=G),
                pq[:, 0 : G * DH].rearrange("p (g n) -> p g n", g=G),
                rsq[:].unsqueeze(-1).broadcast_to([P, G, DH]),
                op=MULT,
            )
            qv = qkn[:].rearrange("p (g n) -> p g n", g=G)
            cb = cos2_sb[:, iA, :].unsqueeze(1).broadcast_to([P, G, DH])
            sb = sin2_sb[:, iA, :].unsqueeze(1).broadcast_to([P, G, DH])
            ta = work.tile([P, G * DH], BF16, tag="ta")
            tb = work.tile([P, G * DH], BF16, tag="tb")
            tav = ta[:].rearrange("p (g n) -> p g n", g=G)
            tbv = tb[:].rearrange("p (g n) -> p g n", g=G)
            nc.vector.tensor_tensor(tav, qv, cb, op=MULT)
            nc.vector.tensor_tensor(tbv, qv, sb, op=MULT)
            rp = work.tile([P, G * DH], BF16, tag="rp")
            rv = rp[:].rearrange("p (g n) -> p g n", g=G)
            nc.vector.tensor_tensor(
                rv[:, :, 0:32], tav[:, :, 0:32], tbv[:, :, 32:64], op=SUB
            )
            nc.vector.tensor_tensor(
                rv[:, :, 32:64], tbv[:, :, 0:32], tav[:, :, 32:64], op=ADD
            )
            rp_t[iA] = rp

        # ================= Pool =================
        if 0 <= t3 < NT:
            rcb = attn.tile([64, 512], BF16, name=f"rcb_{t3}", tag="rcb")
            nc.gpsimd.partition_broadcast(rcb[:], rc_neg_t[t3][:])
            rcb_t[t3] = rcb
        if 0 <= t1 < NT:
            bl = blocks_of(t1)
            j0, nb = bl[0][1], len(bl)
            em = attn.tile([P, 4, 512], BF16, name=f"em_{t1}", tag="em")
            mb = band_tiles[t1]
            nc.vector.tensor_tensor(
                em[:, 0:nb, :].rearrange("p b (h q) -> p b h q", h=HPC),
                ex_t[t1][:, j0 : j0 + nb, :].rearrange(
                    "p b (h q) -> p b h q", h=HPC
                ),
                mb[:, j0 : j0 + nb, :].unsqueeze(2).broadcast_to([P, nb, HPC, P]),
                op=MULT,
            )
            em_t[t1] = em
            ex_t[t1] = None

        # ================= Act (tail): qkT copies =================
        if 0 <= iT < NT:
            qkT = qkT_t[iT]
            qt_v = qkt_all[:, iT, 0 : 4 * P].rearrange("p (h q) -> p h q", h=HPC)
            nc.scalar.copy(qt_v[:, 0::2, :], qkT[0:64, 0:2, :])
            nc.scalar.copy(qt_v[:, 1::2, :], qkT[64:128, 0:2, :])
            nc.scalar.copy(qkt_all[:, iT, 4 * P : 5 * P], qkT[0:64, 2, :])
            qkT_t[iT] = None
        if 0 <= t3 < NT:
            ub = den_t[t3]
            rcb = rcb_t[t3]
            sc_, qoff = t3 // 4, (t3 % 4) * P
            for half in range(2):
                nc.vector.scalar_tensor_tensor(
                    ctxt[sc_][64 * half : 64 * half + 64, :, qoff : qoff + P],
                    ub[:].rearrange("p (h q) -> p h q", h=HPC)[
                        :, half::2, :
                    ],
                    -1.0,
                    rcb[:].rearrange("p (h q) -> p h q", h=HPC)[:, half::2, :],
                    op0=MULT,
                    op1=MULT,
                )
            den_t[t3] = None
            rcb_t[t3] = None

    # ---------------- Phase C (sc=3; 0..2 ran inside the loop tail) ------
    emit_C(3)


def build_program():
    nc = bacc.Bacc("TRN2", target_bir_lowering=False, debug=False, num_devices=8)
    d = {}
    d["xT"] = nc.dram_tensor("xT", [DM, S], BF16, kind="ExternalInput").ap()
    d["wqkv"] = nc.dram_tensor("wqkv", [DM, 384], BF16, kind="ExternalInput").ap()
    d["wo"] = nc.dram_tensor("wo", [256, DM], BF16, kind="ExternalInput").ap()
    d["cos2"] = nc.dram_tensor("cos2", [S, DH], BF16, kind="ExternalInput").ap()
    d["sin2"] = nc.dram_tensor("sin2", [S, DH], BF16, kind="ExternalInput").ap()
    d["band"] = nc.dram_tensor("band", [NT, P, 4, P], BF16, kind="ExternalInput").ap()
    d["gden"] = nc.dram_tensor("gden", [NT, 512], BF16, kind="ExternalInput").ap()
    d["outT"] = nc.dram_tensor("outT", [DM, S], BF16, kind="ExternalOutput").ap()
    with tile.TileContext(nc) as tc, ExitStack() as ctx:
        _build_kernel(ctx, tc, d)
    nc.compile()
    return nc


def make_masks(mask_np):
    """Band tiles [k,q]-oriented: blocks j=0..2 are k-tiles t-2+j, block 3 is
    the global block (k-tile 0, rows >= NGLOB zero, only used for t>=3)."""
    mask_np = np.asarray(mask_np).astype(bool)
    q = np.arange(S)[:, None]
    kk = np.arange(S)[None, :]
    wmask = ((kk <= q) & (kk > q - WINDOW)) | (kk < NGLOB)
    comb = mask_np[0, 0] & wmask  # [q, k]
    combT = comb.T.astype(np.float32)  # [k, q]
    band = np.zeros((NT, P, 4, P), np.float32)
    for t in range(NT):
        for kt in range(max(0, t - 2), t + 1):
            j = kt - (t - 2)
            band[t, :, j, :] = combT[P * kt : P * (kt + 1), P * t : P * (t + 1)]
        if t >= 3:
            band[t, 0:NGLOB, 3, :] = combT[0:NGLOB, P * t : P * (t + 1)]
    # 1/n(q) per (tile, h*q): the Newton seed for the softmax denominator
    n = comb.sum(axis=1).astype(np.float64)  # attended count per q row
    n = np.maximum(n, 1.0)
    gd = (1.0 / n).reshape(NT, P)
    gden = np.repeat(gd[:, None, :], HPC, axis=1).reshape(NT, 512)
    return band, gden


def make_in_maps(x, cos, sin, mask, Wq, Wk, Wv, Wo):
    import ml_dtypes

    bf = ml_dtypes.bfloat16
    x = np.asarray(x, np.float32)
    cos = np.asarray(cos, np.float32)
    sin = np.asarray(sin, np.float32)
    cos2 = np.concatenate([cos, cos], axis=1).astype(bf)
    sin2 = np.concatenate([sin, sin], axis=1).astype(bf)
    Wq, Wk, Wv = (np.asarray(a, np.float32) for a in (Wq, Wk, Wv))
    Wo = np.asarray(Wo, np.float32).astype(bf)
    band, gden = make_masks(mask)
    band = band.astype(bf)
    gden = gden.astype(bf)
    xT = [np.ascontiguousarray(x[b].T).astype(bf) for b in range(B)]
    in_maps = []
    for c in range(8):
        b, g = divmod(c, 4)
        wqkv = np.concatenate(
            [
                Wq[:, 256 * g : 256 * (g + 1)],
                Wk[:, 64 * g : 64 * (g + 1)],
                Wv[:, 64 * g : 64 * (g + 1)],
            ],
            axis=1,
        ).astype(bf)
        in_maps.append(
            {
                "xT": xT[b],
                "wqkv": np.ascontiguousarray(wqkv),
                "wo": np.ascontiguousarray(Wo[256 * g : 256 * (g + 1), :]),
                "cos2": cos2,
                "sin2": sin2,
                "band": band,
                "gden": gden,
            }
        )
    return in_maps


_PROGRAM = None


def _get_program():
    global _PROGRAM
    if _PROGRAM is None:
        _PROGRAM = build_program()
    return _PROGRAM


def kernel(x, cos, sin, mask, Wq, Wk, Wv, Wo, _trace=False, _trace_kwargs=None):
    nc = _get_program()
    in_maps = make_in_maps(x, cos, sin, mask, Wq, Wk, Wv, Wo)
    res = run_bass_kernel_spmd(
        nc, in_maps, list(range(8)), trace=_trace, **(_trace_kwargs or {})
    )
    out = np.zeros((B, S, DM), np.float32)
    for c in range(8):
        out[c // 4] += res.results[c]["outT"].T.astype(np.float32)
    if _trace:
        kernel._last_results = res
    return out


# revision 21
# speedup vs baseline: 2.4337x; 1.0593x over previous
"""Trainium2 Bass kernel for GroupedQueryAttention (sliding-window + global).

Sharding: 8 cores = 2 (batch) x 4 (GQA groups). Core c handles batch c//4 and
kv-head g=c%4 with its 4 query heads. Wq/Wk/Wv column-sharded, Wo row-sharded;
each core emits outT = (context_g @ Wo_g)^T in bf16; the host transposes,
upcasts and sums partials per batch.

v3 design notes:
- Host pre-transposes and pre-casts x to bf16 (xT): no device-side x
  transposes or f32->bf16 casts.
- Scalar engine uses only {Exp, Square, Copy} (all live in the exp
  activation table): zero ACT_TABLE_LOAD swaps. The L2-norm rsqrt and the
  softmax-denominator reciprocal run as DVE Newton iterations (bit-trick
  seed for rsqrt; host-provided 1/attended-count seed for the denominator,
  avoiding the ~3.3us hardware RECIPROCAL).
- Deep software pipeline: iter k = QKV(k) | transposes(k-1) |
  scores+exp+mask(k-2) | ctx(k-3) | recip+divide(k-4), giving every
  cross-engine dependency >= 1 iteration of slack so the tensor engine
  streams continuously and ramps to its fast p-state.
- PSUM budget: qkv tag 2 banks + one 4-bank score tile + ctx 2 banks = 8.
"""

import sys

for _p in (
    "/opt/trn_rl_repo",
    "/root/.axon_site",
    "/root/.axon_site/_ro/pypackages",
    "/root/.axon_site/_ro/trn_rl_repo",
):
    if _p not in sys.path:
        sys.path.insert(0, _p)

from contextlib import ExitStack

import numpy as np

import concourse.bass as bass  # noqa: F401  (registers engine classes)
import concourse.tile as tile
from concourse import bacc, mybir
from concourse.bass_utils import run_bass_kernel_spmd
from concourse.masks import make_identity

B, S, DM = 2, 2048, 1024
NH, NKV, DH = 16, 4, 64
HPC = 4
WINDOW, NGLOB = 256, 4
SCALE = 1.0 / np.sqrt(DH)
CAP = 15.0
EPS = 1e-8
P = 128
NT = S // P
G = HPC + 1
F32 = mybir.dt.float32
U32 = mybir.dt.uint32
BF16 = mybir.dt.bfloat16
MULT = mybir.AluOpType.mult
ADD = mybir.AluOpType.add
SUB = mybir.AluOpType.subtract
EXP = mybir.ActivationFunctionType.Exp
RSQRT_MAGIC = float(np.frombuffer(np.uint32(0x5F3759DF).tobytes(), np.float32)[0])


def _build_kernel(ctx, tc, d):
    nc = tc.nc

    consts = ctx.enter_context(tc.tile_pool(name="consts", bufs=1))
    ident_bf = consts.tile([P, P], BF16)
    identf = consts.tile([P, P], F32)
    make_identity(nc, identf[:])
    nc.vector.tensor_copy(ident_bf[:], identf[:])
    magic = consts.tile([P, 1], F32)
    nc.vector.memset(magic[:], RSQRT_MAGIC)

    wqkv_sb = consts.tile([P, 8, 384], BF16)
    wo_sb = consts.tile([P, 2, DM], BF16)
    cos2_sb = consts.tile([P, NT, DH], BF16)
    sin2_sb = consts.tile([P, NT, DH], BF16)
    g_sb = consts.tile([1, NT, 512], BF16)

    xts = consts.tile([P, 8, S], BF16)
    qkt_all = consts.tile([64, NT, 5 * P], BF16)  # 4 q heads then k, per tile
    v_all = consts.tile([P, NT, 65], BF16)
    ctxt = [consts.tile([P, 2, 512], BF16, name=f"ctxt_{sc}") for sc in range(4)]

    for i in range(NT):
        nc.gpsimd.memset(v_all[:, i, 64:65], 1.0)

    work = ctx.enter_context(tc.tile_pool(name="work", bufs=3))
    attn = ctx.enter_context(tc.tile_pool(name="attn", bufs=3))
    denp = ctx.enter_context(tc.tile_pool(name="denp", bufs=3))
    outp = ctx.enter_context(tc.tile_pool(name="outp", bufs=4))
    mbp = ctx.enter_context(tc.tile_pool(name="mbp", bufs=4))

    ps = ctx.enter_context(tc.tile_pool(name="ps", bufs=1, space="PSUM"))

    def dma_x(i):
        if 0 <= i < NT:
            nc.sync.dma_start(
                xts[:, :, P * i : P * (i + 1)],
                d["xT"].rearrange("(c p) s -> p c s", p=P)[:, :, P * i : P * (i + 1)],
            )

    band_tiles = [None] * NT

    def dma_band(t):
        if 0 <= t < NT:
            mb = mbp.tile([P, 4, P], BF16, name=f"mb_{t}", tag="mb")
            nc.sync.dma_start(mb[:], d["band"][t])
            band_tiles[t] = mb

    def blocks_of(t):
        bl = [(kt, kt - (t - 2)) for kt in range(max(0, t - 2), t + 1)]
        if t >= 3:
            bl.append((0, 3))
        return bl

    qkv_ps = [None] * NT
    ssq_t = [None] * NT
    rp_t = [None] * NT
    qkT_t = [None] * NT
    ex_t = [None] * NT
    em_t = [None] * NT
    sc_ps_t = [None] * NT
    pcx_t = [None] * NT
    den_t = [None] * NT
    dn_t = [None] * NT
    rc_neg_t = [None] * NT
    rcb_t = [None] * NT

    def emit_C(sc):
        for mo in range(8):
            po = ps.tile([P, 512], F32, name=f"po_{sc}_{mo}", tag="qkv", bufs=2,
                         padded_shape=[P, 512])
            for c in range(2):
                nc.tensor.matmul(
                    po[:],
                    lhsT=wo_sb[:, c, P * mo : P * (mo + 1)],
                    rhs=ctxt[sc][:, c, :],
                    start=(c == 0),
                    stop=(c == 1),
                )
            ob = outp.tile([P, 512], BF16, tag="ob")
            if mo % 2 == 1:
                nc.scalar.copy(ob[:], po[:])
            else:
                nc.vector.tensor_copy(ob[:], po[:])
            nc.sync.dma_start(
                d["outT"][P * mo : P * (mo + 1), 512 * sc : 512 * (sc + 1)], ob[:]
            )

    dma_x(0)
    nc.sync.dma_start(wqkv_sb[:], d["wqkv"].rearrange("(c p) n -> p c n", p=P))
    nc.sync.dma_start(cos2_sb[:], d["cos2"].rearrange("(t p) n -> p t n", p=P))
    nc.sync.dma_start(sin2_sb[:], d["sin2"].rearrange("(t p) n -> p t n", p=P))
    dma_x(1)

    for k in range(NT + 4):
        iA = k       # QKV + norm + rope
        iT = k - 1   # transposes + qkT copies
        t1 = k - 2   # scores + exp + mask
        t2 = k - 3   # ctx matmuls + den extract
        t3 = k - 4   # reciprocal + divide

        dma_x(iA + 2)
        dma_band(k)
        if k == 1:
            nc.sync.dma_start(g_sb[:], d["gden"].unsqueeze(0))
        if k == 8:
            nc.sync.dma_start(wo_sb[:], d["wo"].rearrange("(c p) n -> p c n", p=P))

        # ================= PE =================
        if iA < NT:
            pq = ps.tile([P, 384], F32, name=f"pq_{iA}", tag="qkv", bufs=2,
                         padded_shape=[P, 512])
            for mj in range(8):
                nc.tensor.matmul(
                    pq[:],
                    lhsT=xts[:, mj, P * iA : P * (iA + 1)],
                    rhs=wqkv_sb[:, mj, :],
                    start=(mj == 0),
                    stop=(mj == 7),
                )
            qkv_ps[iA] = pq

        if 0 <= t1 < NT:
            bl = blocks_of(t1)
            qrhs = qkt_all[:, t1, 0 : 4 * P].rearrange("p (h q) -> p h q", h=HPC)
            passes = []
            for pi in range(2):
                blkpass = [b for b in bl if (b[1] < 2) == (pi == 0)]
                if not blkpass:
                    continue
                scp = ps.tile(
                    [P, 2, 512], F32, name=f"sc_{t1}_{pi}", tag="sc", bufs=2
                )
                for bj, (kt, j) in enumerate(blkpass):
                    nc.tensor.matmul(
                        scp[:, bj, :],
                        lhsT=qkt_all[:, kt, 4 * P : 4 * P + P],
                        rhs=qrhs,
                        start=True,
                        stop=True,
                    )
                passes.append((scp, blkpass))
            sc_ps_t[t1] = passes

        if 0 <= iT < NT:
            rp = rp_t[iT]
            qkT = ps.tile([P, 3, P], BF16, name=f"qkT_{iT}", tag="qkv", bufs=2,
                          padded_shape=[P, 8, P])
            for hp in range(2):
                nc.tensor.transpose(
                    qkT[:, hp, :], rp[:, P * hp : P * (hp + 1)], ident_bf[:]
                )
            nc.tensor.transpose(qkT[0:64, 2, :], rp[:, 256:320], ident_bf[:])
            qkT_t[iT] = qkT

        if 0 <= t2 < NT:
            n_all = sum(len(bp) for _, bp in em_t[t2])
            pcx = ps.tile([P, 512], F32, name=f"pcx_{t2}", tag="cx", bufs=2)
            bi = 0
            for em, blkpass in em_t[t2]:
                for bj, (kt, j) in enumerate(blkpass):
                    nc.tensor.matmul(
                        pcx[0:65, :],
                        lhsT=v_all[:, kt, :],
                        rhs=em[:, bj, :],
                        start=(bi == 0),
                        stop=(bi == n_all - 1),
                    )
                    bi += 1
            pcx_t[t2] = pcx
            em_t[t2] = None

        if 16 <= k <= 18:
            emit_C(k - 16)

        # ================= Act =================
        if iA < NT:
            pq = qkv_ps[iA]
            ssq = work.tile([P, G * DH], F32, tag="ssq")
            nc.scalar.square(ssq[:], pq[:, 0 : G * DH])
            ssq_t[iA] = ssq
            nc.scalar.copy(v_all[:, iA, 0:64], pq[:, 320:384])
        if 0 <= t1 < NT:
            exs = []
            for pi, (scp, blkpass) in enumerate(sc_ps_t[t1]):
                nb = len(blkpass)
                ex = attn.tile([P, 2, 512], BF16, name=f"ex_{t1}_{pi}", tag="ex")
                nc.scalar.activation(
                    ex[:, 0:nb, :], scp[:, 0:nb, :], EXP, scale=SCALE
                )
                exs.append((ex, blkpass))
            ex_t[t1] = exs
            sc_ps_t[t1] = None
        if 0 <= t2 < NT:
            ub = attn.tile([64, 512], BF16, name=f"ub_{t2}", tag="ub")
            nc.scalar.copy(ub[:], pcx_t[t2][0:64, :])
            den_t[t2] = ub
            dnc = denp.tile([1, 512], BF16, tag="dnc")
            nc.scalar.copy(dnc[:], pcx_t[t2][64:65, :])
            dn_t[t2] = dnc
            pcx_t[t2] = None

        # ================= DVE (part 1) =================
        if 0 <= t3 < NT:
            dn = dn_t[t3][:]  # +den row (bf16, base partition 0)
            g = g_sb[:, t3, :]
            nt_ = denp.tile([1, 512], BF16, tag="nt")
            nc.vector.tensor_tensor(nt_[:], dn, g, op=MULT)
            ny1 = denp.tile([1, 512], BF16, tag="ny1")
            nc.vector.scalar_tensor_tensor(ny1[:], nt_[:], 2.0, g, op0=SUB, op1=MULT)
            nc.vector.tensor_tensor(nt_[:], dn, ny1[:], op=MULT)
            rc = denp.tile([1, 512], BF16, tag="rc")
            nc.vector.scalar_tensor_tensor(
                rc[:], nt_[:], -2.0, ny1[:], op0=SUB, op1=MULT
            )
            rc_neg_t[t3] = rc
        if iA < NT:
            pq = qkv_ps[iA]
            red = work.tile([P, G], F32, tag="red")
            nc.vector.tensor_reduce(
                red[:],
                ssq_t[iA][:].rearrange("p (g n) -> p g n", g=G),
                axis=mybir.AxisListType.X,
                op=ADD,
            )
            yu = work.tile([P, G], U32, tag="yu")
            nc.vector.tensor_scalar(
                yu[:], red[:].bitcast(U32), 1, None,
                op0=mybir.AluOpType.logical_shift_right,
            )
            y0 = work.tile([P, G], F32, tag="y0")
            nc.vector.tensor_tensor(
                y0[:].bitcast(U32),
                magic[:].bitcast(U32).broadcast_to([P, G]),
                yu[:],
                op=SUB,
            )
            ysq = work.tile([P, G], F32, tag="ysq")
            nc.vector.tensor_tensor(ysq[:], y0[:], y0[:], op=MULT)
            nc.vector.tensor_tensor(ysq[:], ysq[:], red[:], op=MULT)
            rsq = work.tile([P, G], F32, tag="rsq")
            nc.vector.scalar_tensor_tensor(
                rsq[:], ysq[:], 3.0, y0[:], op0=SUB, op1=MULT
            )
            nc.vector.tensor_scalar(rsq[:], rsq[:], -0.5, None, op0=MULT)
            qkn = work.tile([P, G * DH], BF16, tag="qkn")
            nc.vector.tensor_tensor(
                qkn[:].rearrange("p (g n) -> p g n", g=G),
                pq[:, 0 : G * DH].rearrange("p (g n) -> p g n", g=G),
                rsq[:].unsqueeze(-1).broadcast_to([P, G, DH]),
                op=MULT,
            )
            qv = qkn[:].rearrange("p (g n) -> p g n", g=G)
            cb = cos2_sb[:, iA, :].unsqueeze(1).broadcast_to([P, G, DH])
            sb = sin2_sb[:, iA, :].unsqueeze(1).broadcast_to([P, G, DH])
            ta = work.tile([P, G * DH], BF16, tag="ta")
            tb = work.tile([P, G * DH], BF16, tag="tb")
            tav = ta[:].rearrange("p (g n) -> p g n", g=G)
            tbv = tb[:].rearrange("p (g n) -> p g n", g=G)
            nc.vector.tensor_tensor(tav, qv, cb, op=MULT)
            nc.vector.tensor_tensor(tbv, qv, sb, op=MULT)
            rp = work.tile([P, G * DH], BF16, tag="rp")
            rv = rp[:].rearrange("p (g n) -> p g n", g=G)
            nc.vector.tensor_tensor(
                rv[:, :, 0:32], tav[:, :, 0:32], tbv[:, :, 32:64], op=SUB
            )
            nc.vector.tensor_tensor(
                rv[:, :, 32:64], tbv[:, :, 0:32], tav[:, :, 32:64], op=ADD
            )
            rp_t[iA] = rp

        # ================= Pool =================
        if 0 <= t3 < NT:
            rcb = attn.tile([64, 512], BF16, name=f"rcb_{t3}", tag="rcb")
            nc.gpsimd.partition_broadcast(rcb[:], rc_neg_t[t3][:])
            rcb_t[t3] = rcb
        if 0 <= t1 < NT:
            mb = band_tiles[t1]
            ems = []
            for pi, (ex, blkpass) in enumerate(ex_t[t1]):
                nb = len(blkpass)
                j0 = blkpass[0][1]
                em = attn.tile([P, 2, 512], BF16, name=f"em_{t1}_{pi}", tag="em")
                nc.vector.tensor_tensor(
                    em[:, 0:nb, :].rearrange("p b (h q) -> p b h q", h=HPC),
                    ex[:, 0:nb, :].rearrange("p b (h q) -> p b h q", h=HPC),
                    mb[:, j0 : j0 + nb, :]
                    .unsqueeze(2)
                    .broadcast_to([P, nb, HPC, P]),
                    op=MULT,
                )
                ems.append((em, blkpass))
            em_t[t1] = ems
            ex_t[t1] = None

        # ================= Act (tail): qkT copies =================
        if 0 <= iT < NT:
            qkT = qkT_t[iT]
            qt_v = qkt_all[:, iT, 0 : 4 * P].rearrange("p (h q) -> p h q", h=HPC)
            nc.scalar.copy(qt_v[:, 0::2, :], qkT[0:64, 0:2, :])
            nc.scalar.copy(qt_v[:, 1::2, :], qkT[64:128, 0:2, :])
            nc.scalar.copy(qkt_all[:, iT, 4 * P : 5 * P], qkT[0:64, 2, :])
            qkT_t[iT] = None
        if 0 <= t3 < NT:
            ub = den_t[t3]
            rcb = rcb_t[t3]
            sc_, qoff = t3 // 4, (t3 % 4) * P
            for half in range(2):
                nc.vector.scalar_tensor_tensor(
                    ctxt[sc_][64 * half : 64 * half + 64, :, qoff : qoff + P],
                    ub[:].rearrange("p (h q) -> p h q", h=HPC)[
                        :, half::2, :
                    ],
                    -1.0,
                    rcb[:].rearrange("p (h q) -> p h q", h=HPC)[:, half::2, :],
                    op0=MULT,
                    op1=MULT,
                )
            den_t[t3] = None
            rcb_t[t3] = None

    # ---------------- Phase C (sc=3; 0..2 ran inside the loop tail) ------
    emit_C(3)


def build_program():
    nc = bacc.Bacc("TRN2", target_bir_lowering=False, debug=False, num_devices=8)
    d = {}
    d["xT"] = nc.dram_tensor("xT", [DM, S], BF16, kind="ExternalInput").ap()
    d["wqkv"] = nc.dram_tensor("wqkv", [DM, 384], BF16, kind="ExternalInput").ap()
    d["wo"] = nc.dram_tensor("wo", [256, DM], BF16, kind="ExternalInput").ap()
    d["cos2"] = nc.dram_tensor("cos2", [S, DH], BF16, kind="ExternalInput").ap()
    d["sin2"] = nc.dram_tensor("sin2", [S, DH], BF16, kind="ExternalInput").ap()
    d["band"] = nc.dram_tensor("band", [NT, P, 4, P], BF16, kind="ExternalInput").ap()
    d["gden"] = nc.dram_tensor("gden", [NT, 512], BF16, kind="ExternalInput").ap()
    d["outT"] = nc.dram_tensor("outT", [DM, S], BF16, kind="ExternalOutput").ap()
    with tile.TileContext(nc) as tc, ExitStack() as ctx:
        _build_kernel(ctx, tc, d)
    nc.compile()
    return nc


def make_masks(mask_np):
    """Band tiles [k,q]-oriented: blocks j=0..2 are k-tiles t-2+j, block 3 is
    the global block (k-tile 0, rows >= NGLOB zero, only used for t>=3)."""
    mask_np = np.asarray(mask_np).astype(bool)
    q = np.arange(S)[:, None]
    kk = np.arange(S)[None, :]
    wmask = ((kk <= q) & (kk > q - WINDOW)) | (kk < NGLOB)
    comb = mask_np[0, 0] & wmask  # [q, k]
    combT = comb.T.astype(np.float32)  # [k, q]
    band = np.zeros((NT, P, 4, P), np.float32)
    for t in range(NT):
        for kt in range(max(0, t - 2), t + 1):
            j = kt - (t - 2)
            band[t, :, j, :] = combT[P * kt : P * (kt + 1), P * t : P * (t + 1)]
        if t >= 3:
            band[t, 0:NGLOB, 3, :] = combT[0:NGLOB, P * t : P * (t + 1)]
    # 1/n(q) per (tile, h*q): the Newton seed for the softmax denominator
    n = comb.sum(axis=1).astype(np.float64)  # attended count per q row
    n = np.maximum(n, 1.0)
    gd = (1.0 / n).reshape(NT, P)
    gden = np.repeat(gd[:, None, :], HPC, axis=1).reshape(NT, 512)
    return band, gden


def make_in_maps(x, cos, sin, mask, Wq, Wk, Wv, Wo):
    import ml_dtypes

    bf = ml_dtypes.bfloat16
    x = np.asarray(x, np.float32)
    cos = np.asarray(cos, np.float32)
    sin = np.asarray(sin, np.float32)
    cos2 = np.concatenate([cos, cos], axis=1).astype(bf)
    sin2 = np.concatenate([sin, sin], axis=1).astype(bf)
    Wq, Wk, Wv = (np.asarray(a, np.float32) for a in (Wq, Wk, Wv))
    Wo = np.asarray(Wo, np.float32).astype(bf)
    band, gden = make_masks(mask)
    band = band.astype(bf)
    gden = gden.astype(bf)
    xT = [np.ascontiguousarray(x[b].T).astype(bf) for b in range(B)]
    in_maps = []
    for c in range(8):
        b, g = divmod(c, 4)
        wqkv = np.concatenate(
            [
                Wq[:, 256 * g : 256 * (g + 1)],
                Wk[:, 64 * g : 64 * (g + 1)],
                Wv[:, 64 * g : 64 * (g + 1)],
            ],
            axis=1,
        ).astype(bf)
        in_maps.append(
            {
                "xT": xT[b],
                "wqkv": np.ascontiguousarray(wqkv),
                "wo": np.ascontiguousarray(Wo[256 * g : 256 * (g + 1), :]),
                "cos2": cos2,
                "sin2": sin2,
                "band": band,
                "gden": gden,
            }
        )
    return in_maps


_PROGRAM = None


def _get_program():
    global _PROGRAM
    if _PROGRAM is None:
        _PROGRAM = build_program()
    return _PROGRAM


def kernel(x, cos, sin, mask, Wq, Wk, Wv, Wo, _trace=False, _trace_kwargs=None):
    nc = _get_program()
    in_maps = make_in_maps(x, cos, sin, mask, Wq, Wk, Wv, Wo)
    res = run_bass_kernel_spmd(
        nc, in_maps, list(range(8)), trace=_trace, **(_trace_kwargs or {})
    )
    out = np.zeros((B, S, DM), np.float32)
    for c in range(8):
        out[c // 4] += res.results[c]["outT"].T.astype(np.float32)
    if _trace:
        kernel._last_results = res
    return out
